# revision 1
# baseline (speedup 1.0000x reference)
"""CCNF RK4 sampling kernel for 8 Trainium2 NeuronCores.

Data-parallel: batch 2048 -> 256 per core, weights replicated.
On-core layout: features on partitions, batch on the free dim (N=256).
Layer 1/3 matmuls in bf16 (1 cyc/row); layer 2 — the FLOP bulk — in
fp8-e4m3 with DoubleRow perf mode (0.5 cyc/row AND K=256 packed per
matmul: 16 pair-MMs replace 32 bf16 MMs, 4x less PE time). Measured
rel err 8.4e-03 vs the 2e-2 gate (numpy-probed first: fp8 L2 alone
contributes ~8e-3; bf16 everywhere was 9.4e-4).

Over the v1 baseline (463066 -> 440911 ns, cost model):
  - fp8 DoubleRow layer 2: w2 shipped as [128, pair, k-plane, cols],
    h1 GLU outputs written as fp8 pair tiles [128, 2, 256] that are the
    DoubleRow moving operand directly.
  - t-row folded into a host-precomputed per-t bias table (33 distinct
    t values), removing the per-eval memset from the serial RK4 chain.
  - bf16/fp8 weights and activations shrink the weight DMA footprint.
  - startup DMA chain minimized: HWDGE prep is 625ns/DMA (serialized)
    and completion-sem propagation 900ns, so the L1-critical tensors
    ship as two packed DMAs ([ctx | W1-ctx-b | W1-ctx-a] and
    [theta0 | W1-theta]), w2 streams per k-chunk, bias tables ride one
    DMA, and the unused ones/b3 constant is skipped when b3 == 0.
  - L3 shares the 8-bank PSUM ring (no dedicated bank).

Per-eval steady state (cost model): PE 3.0us (43% busy) — the eval is
now latency-bound on the sigmoid->GLU pipeline (DVE 58%, ACT 47%; 8
chunks x ~590ns ACT/DVE cadence plus the RK4 theta-update tail).
Next lever if revisited: pair adjacent PSUM banks into [128,2,2,256]
tiles so sigma/GLU run 4 double-width ops (needs biases moved into
matmuls via a ones-row, using the idle PE). Measured dead ends (all
worse): 128-column stream splits (per-op ACT/DVE overheads exceed the
latency hidden), kc-rotation in L2 groups (removes scheduler freedom),
PSUM pre-fill / filler shuffling (wall = work + exposed chain is
invariant under any reordering the scheduler can already do).
"""

import os

import numpy as np
from ml_dtypes import bfloat16 as _bf16
from ml_dtypes import float8_e4m3 as _f8np

N_CORES = 8


def _build_program(theta0, context, W1, b1, W2, b2, W3, b3, n_steps):
    import concourse.bass as bass
    import concourse.mybir as mybir
    import concourse.tile as tile
    from concourse import bacc

    f32 = mybir.dt.float32
    f32r = mybir.dt.float32r
    bf16 = mybir.dt.bfloat16
    f8 = mybir.dt.float8e4
    DR = mybir.MatmulPerfMode.DoubleRow
    ALU = mybir.AluOpType
    SIGMOID = mybir.ActivationFunctionType.Sigmoid

    B, D = theta0.shape          # 2048, 32
    C = context.shape[1]         # 128
    IN, H2 = W1.shape            # 161, 1024
    H = W2.shape[0]              # 512
    assert H2 == 2 * H and W2.shape[1] == 2 * H and W3.shape == (H, D)
    assert IN == D + 1 + C
    assert B % N_CORES == 0
    Bs = B // N_CORES            # 256 per core
    steps = int(n_steps)
    dt = 1.0 / steps

    KC = H // 128                # 4 k-chunks for layer 2/3
    MJ = H // 128                # 4 output column-chunks per GLU half
    K1B = C                      # 128 ctx rows

    # ---- host-side layout prep (shared across cores) ----
    W1 = np.asarray(W1, np.float32)
    w1th_h = np.ascontiguousarray(W1[:D])                      # [32, 1024] theta rows
    w1t_row = W1[D]                                            # [1024] time row
    w1c2_h = np.ascontiguousarray(W1[D + 1 :])                 # [128, 1024] ctx rows
    b1 = np.asarray(b1, np.float32)
    b2 = np.asarray(b2, np.float32)
    b3_is_zero = not np.any(np.asarray(b3, np.float32))
    # DoubleRow layout: [512,1024] -> [128, pair(2), plane(2), 1024]
    # (pair P covers kc = 2P, 2P+1; plane i is the kc = 2P+i k-tile)
    KCP = KC // 2
    w2_h = np.ascontiguousarray(
        np.asarray(W2, np.float32)
        .reshape(KCP, 2, 128, 2 * H).transpose(2, 0, 1, 3)
        .reshape(128, KCP * 2 * 2 * H)
    )
    # [512,32] -> [128, 4*32]
    w3_h = np.ascontiguousarray(
        np.asarray(W3, np.float32).reshape(KC, 128, D).transpose(1, 0, 2).reshape(128, KC * D)
    )
    # per-t layer-1 bias table: bias(t) = b1 + t * w1t_row, t = k*dt/2 for
    # k = 0..2*steps. layout per t: [a-half j cols | b-half j cols] = [128, 8]
    NT = 2 * steps + 1
    tvals = (np.arange(NT, dtype=np.float32) * (dt / 2.0)).reshape(NT, 1)
    btab = b1.reshape(1, 2 * H) + tvals * w1t_row.reshape(1, 2 * H)  # [NT, 1024]
    btab = btab.reshape(NT, 2, MJ, 128)                              # (t, half, j, p)
    bias_t_h = np.ascontiguousarray(
        btab.transpose(3, 0, 1, 2).reshape(128, NT * 2 * MJ)
    )                                                          # [128, NT*8]
    bias2_h = np.concatenate([
        b2[:H].reshape(MJ, 128).T, b2[H:].reshape(MJ, 128).T,
    ], axis=1)                                                 # [128, 8]
    bias_t_h = np.ascontiguousarray(
        np.concatenate([bias_t_h, bias2_h], axis=1)
    )                                                          # [128, NT*8+8]
    ctxpack_h = None  # built per-core (contains the ctx shard)
    onesb3_h = np.ascontiguousarray(np.concatenate([
        np.ones((1, Bs), np.float32),
        np.asarray(b3, np.float32).reshape(1, D),
    ], axis=1))                                                # [1, Bs+32]
    # onesb3 only ships when b3 is nonzero (it is zero for this problem)

    # ---- build the bass program (same program on all 8 cores) ----
    nc = bacc.Bacc("TRN2", target_bir_lowering=False)

    d_th0 = nc.dram_tensor("th0", [D, Bs], f32r, kind="ExternalInput")
    # packed: [x2 (Bs) | w1c2 b-half (H) | w1c2 a-half (H)] — one partition
    # group, two DMAs (critical prefix first), one HWDGE prep each
    d_ctxpack = nc.dram_tensor("ctxpack", [K1B, Bs + 2 * H], bf16,
                               kind="ExternalInput")
    # packed: [th0h (Bs) | w1th (2H)]
    d_thpack = nc.dram_tensor("thpack", [D, Bs + 2 * H], bf16,
                              kind="ExternalInput")
    d_w2 = nc.dram_tensor("w2", [128, KCP * 2 * 2 * H], f8, kind="ExternalInput")
    d_w3 = nc.dram_tensor("w3", [128, KC * D], bf16, kind="ExternalInput")
    d_biast = nc.dram_tensor("biast", [128, NT * 2 * MJ + 2 * MJ], bf16, kind="ExternalInput")
    d_ob3 = (None if b3_is_zero else
             nc.dram_tensor("onesb3", [1, Bs + D], f32r, kind="ExternalInput"))
    d_out = nc.dram_tensor("out", [D, Bs], f32, kind="ExternalOutput")

    # RK4 coefficients: arg scale (for next eval's input), acc scale
    c_arg = [0.5 * dt, 0.5 * dt, dt]
    a_acc = [dt / 6.0, dt / 3.0, dt / 3.0, dt / 6.0]
    # t index per (step, eval): t = (s + TOFF[e]) * dt -> idx = 2s + IOFF[e]
    IOFF = (0, 1, 1, 2)

    FSPLIT = int(os.environ.get("KERNEL_FSPLIT", "2"))  # ctx banks in stall-1 slot

    with tile.TileContext(nc) as tc:
        with (
            tc.tile_pool(name="const", bufs=1) as cpool,
            tc.tile_pool(name="psmm", bufs=8, space="PSUM") as ps_pool,
            tc.tile_pool(name="sig", bufs=int(os.environ.get("KERNEL_SIGB", "10"))) as sig_pool,
            tc.tile_pool(name="hact", bufs=int(os.environ.get("KERNEL_HB", "20"))) as h_pool,
            tc.tile_pool(name="accp", bufs=int(os.environ.get("KERNEL_AB", "6"))) as acc_pool,
        ):
            tctx = cpool.tile([K1B, Bs + 2 * H], bf16)
            tx2 = tctx[:, 0:Bs]
            # w1c2 columns: b-half at [Bs : Bs+H], a-half at [Bs+H : Bs+2H]
            tthp = cpool.tile([D, Bs + 2 * H], bf16)
            tx1 = tthp[:, 0:Bs]
            tw1th = tthp[:, Bs : Bs + 2 * H]
            tw2 = cpool.tile([128, KCP, 2, 2 * H], f8)
            tw3 = cpool.tile([128, KC * D], bf16)
            tbiast = cpool.tile([128, NT * 2 * MJ + 2 * MJ], bf16)
            tb2a = tbiast[:, NT * 2 * MJ : NT * 2 * MJ + MJ]
            tb2b = tbiast[:, NT * 2 * MJ + MJ : NT * 2 * MJ + 2 * MJ]
            if not b3_is_zero:
                tob3 = cpool.tile([1, Bs + D], f32r)
                tones = tob3[:, 0:Bs]
                tb3 = tob3[:, Bs : Bs + D]
            tth0 = cpool.tile([D, Bs], f32r)    # initial theta

            def w1c2_col(mj):
                # mj >= MJ: b-half chunk, else a-half chunk
                if mj >= MJ:
                    base = Bs + (mj - MJ) * 128
                else:
                    base = Bs + H + mj * 128
                return tctx[:, base : base + 128]

            def tb1a(idx, j):
                return tbiast[:, idx * 2 * MJ + j : idx * 2 * MJ + j + 1]

            def tb1b(idx, j):
                return tbiast[:, idx * 2 * MJ + MJ + j : idx * 2 * MJ + MJ + j + 1]

            # L1-critical tensors first so eval 0 can start while w2/w3
            # still stream; w2 split per k-chunk so layer 2 can begin
            # before the full weight matrix lands.
            nc.sync.dma_start(tctx[:, 0 : Bs + H], d_ctxpack[:, 0 : Bs + H])
            nc.sync.dma_start(tctx[:, Bs + H : Bs + 2 * H],
                              d_ctxpack[:, Bs + H : Bs + 2 * H])
            nc.sync.dma_start(tthp[:], d_thpack[:])
            nc.sync.dma_start(tbiast[:], d_biast[:])
            for P in range(KCP):
                nc.sync.dma_start(
                    tw2[:, P, :, :],
                    d_w2[:, P * 2 * 2 * H : (P + 1) * 2 * 2 * H],
                )
                if P == 0:
                    nc.sync.dma_start(tth0[:], d_th0[:])
            nc.sync.dma_start(tw3[:], d_w3[:])
            if not b3_is_zero:
                nc.sync.dma_start(tob3[:], d_ob3[:])

            def mm(out_ap, lhsT_ap, rhs_ap, start, stop, pm=None):
                nc.tensor.matmul(out_ap, lhsT_ap, rhs_ap, start=start,
                                 stop=stop, perf_mode=pm)

            th_cur = tth0       # theta at start of current step

            def issue_l1ctx(js):
                # static context contribution for the NEXT eval's layer 1,
                # placed in PE stall windows. One accumulation group per
                # bank; stop goes on the last theta MM next eval.
                tiles = []
                for j in js:
                    ps = ps_pool.tile([128, 2 * Bs], f32, tag="psmm")
                    for half, mj in ((1, j + MJ), (0, j)):
                        dst = ps[:, half * Bs : (half + 1) * Bs]
                        mm(dst, w1c2_col(mj), tx2[:],
                           start=(half == 1), stop=False)
                    tiles.append(ps)
                return tiles

            # only 3 of 4 L1 banks are pre-issued: 8 bank allocs per eval on
            # the 8-slot ring (a 4th pre-issue would shift slot reuse into
            # live banks and stall ~650ns/eval on bank-free waits). Bank j=3
            # gets its ctx MMs inline, right before its theta MMs.
            ps1 = issue_l1ctx(range(MJ - 1))

            for s in range(steps):
                for e in range(4):
                    idx = 2 * s + IOFF[e]
                    last_eval = (s == steps - 1) and (e == 3)

                    # ---- layer 1: theta MMs close the pre-issued banks ----
                    # h1 chunks land in fp8 pair tiles [128, plane(2), Bs]
                    # (plane = kc within the pair) feeding DoubleRow L2 MMs
                    h1p = [h_pool.tile([128, 2, Bs], f8, tag="h1t",
                                       name=f"h1p{P}")
                           for P in range(KCP)]
                    for j in range(MJ):
                        if j < len(ps1):
                            ps = ps1[j]
                            first = False
                        else:
                            ps = ps_pool.tile([128, 2 * Bs], f32, tag="psmm")
                            first = True
                        for half, mj in ((1, j + MJ), (0, j)):
                            dst = ps[:, half * Bs : (half + 1) * Bs]
                            if first:
                                # start=True only on the bank's first MM: a
                                # second start would re-mark the whole bank
                                # pending-zero and wipe the b-half sums
                                mm(dst, w1c2_col(mj), tx2[:],
                                   start=(half == 1), stop=False)
                        for half, mj in ((1, j + MJ), (0, j)):
                            dst = ps[:, half * Bs : (half + 1) * Bs]
                            msl = slice(mj * 128, (mj + 1) * 128)
                            mm(dst, tw1th[:, msl], tx1[:], start=False,
                               stop=(half == 0))
                        sg = sig_pool.tile([128, Bs], f32, tag="sig1")
                        nc.scalar.activation(
                            sg[:], ps[:, Bs : 2 * Bs], SIGMOID,
                            bias=tb1b(idx, j)
                        )
                        nc.vector.scalar_tensor_tensor(
                            h1p[j // 2][:, j % 2, :], ps[:, 0:Bs],
                            tb1a(idx, j), sg[:],
                            ALU.add, ALU.mult,
                        )

                    # first chunk of next-eval ctx MMs fills the h1[0] wait
                    if not last_eval:
                        ps1_next = issue_l1ctx(range(FSPLIT))

                    # ---- layer 2 ----
                    h2 = []
                    for j in range(MJ):
                        ps = ps_pool.tile([128, 2 * Bs], f32, tag="psmm")
                        # b-half group first so the sigmoid overlaps the
                        # a-half matmuls
                        dstb = ps[:, Bs : 2 * Bs]
                        for P in range(KCP):
                            csl = slice((j + MJ) * 128, (j + MJ + 1) * 128)
                            mm(dstb, tw2[:, P, :, csl], h1p[P][:],
                               start=(P == 0), stop=(P == KCP - 1), pm=DR)
                        sg = sig_pool.tile([128, Bs], f32, tag="sig2")
                        nc.scalar.activation(
                            sg[:], dstb, SIGMOID, bias=tb2b[:, j : j + 1]
                        )
                        dsta = ps[:, 0:Bs]
                        for P in range(KCP):
                            csl = slice(j * 128, (j + 1) * 128)
                            mm(dsta, tw2[:, P, :, csl], h1p[P][:],
                               start=(P == 0), stop=(P == KCP - 1), pm=DR)
                        ht = h_pool.tile([128, Bs], bf16, tag="h2t")
                        nc.vector.scalar_tensor_tensor(
                            ht[:], dsta, tb2a[:, j : j + 1], sg[:],
                            ALU.add, ALU.mult,
                        )
                        h2.append(ht)

                    # ---- layer 3: k = h2 @ W3 (+ b3) in PSUM ----
                    ps3full = ps_pool.tile([128, 2 * Bs], f32, tag="psmm")
                    ps3 = ps3full[0:D, 0:Bs]
                    for kc in range(KC):
                        mm(ps3[:], tw3[:, kc * D : (kc + 1) * D], h2[kc][:],
                           start=(kc == 0), stop=(kc == KC - 1 and b3_is_zero))
                    if not b3_is_zero:
                        mm(ps3[:], tb3[:], tones[:], start=False, stop=True)

                    # remaining pre-issued ctx MMs fill the tx1 wait at the
                    # boundary (bank j=3 is issued inline next eval)
                    if not last_eval:
                        ps1_next += issue_l1ctx(range(FSPLIT, MJ - 1))

                    # ---- RK4 bookkeeping ----
                    base = th_cur if e == 0 else acc_prev
                    if e < 3:
                        # next eval's theta arg (critical: feeds L1)
                        nc.vector.scalar_tensor_tensor(
                            tx1[:], ps3[:], float(c_arg[e]), th_cur[:],
                            ALU.mult, ALU.add,
                        )
                    elif s != steps - 1:
                        # theta_{s+1} straight into the matmul input tile
                        nc.vector.scalar_tensor_tensor(
                            tx1[:], ps3[:], float(a_acc[e]), base[:],
                            ALU.mult, ALU.add,
                        )
                    # accumulator copy (gpsimd can't read PSUM; keep on DVE,
                    # after the critical tx1 update)
                    acc_new = acc_pool.tile([D, Bs], f32, tag="accp")
                    nc.vector.scalar_tensor_tensor(
                        acc_new[:], ps3[:], float(a_acc[e]), base[:],
                        ALU.mult, ALU.add,
                    )
                    acc_prev = acc_new
                    if not last_eval:
                        ps1 = ps1_next

                th_cur = acc_prev  # theta_{s+1}

            nc.sync.dma_start(d_out[:], th_cur[:])

    # ---- per-core input maps ----
    in_maps = []
    for c in range(N_CORES):
        sl = slice(c * Bs, (c + 1) * Bs)
        th_T = np.ascontiguousarray(np.asarray(theta0[sl], np.float32).T)
        ctx_T = np.ascontiguousarray(np.asarray(context[sl], np.float32).T)
        ctxpack = np.ascontiguousarray(np.concatenate([
            ctx_T.astype(_bf16),
            w1c2_h[:, H : 2 * H].astype(_bf16),
            w1c2_h[:, 0:H].astype(_bf16),
        ], axis=1))
        thpack = np.ascontiguousarray(np.concatenate([
            th_T.astype(_bf16), w1th_h.astype(_bf16)
        ], axis=1))
        in_maps.append(
            {
                "th0": th_T,
                "ctxpack": ctxpack,
                "thpack": thpack,
                "w2": w2_h.astype(_f8np),
                "w3": w3_h.astype(_bf16),
                "biast": bias_t_h.astype(_bf16),
                **({} if b3_is_zero else {"onesb3": onesb3_h}),
            }
        )

    return nc, in_maps


def _build_and_run(theta0, context, W1, b1, W2, b2, W3, b3, n_steps):
    from concourse.bass_utils import run_bass_kernel_spmd

    nc, in_maps = _build_program(theta0, context, W1, b1, W2, b2, W3, b3, n_steps)
    nc.finalize()  # Bacc: split multi-sem waits + allocate registers
    res = run_bass_kernel_spmd(
        nc,
        in_maps,
        core_ids=list(range(N_CORES)),
        trace=bool(int(os.environ.get("KERNEL_TRACE", "0"))),
    )
    _build_and_run.last_results = res

    out = np.concatenate([r["out"].T for r in res.results], axis=0)
    return np.ascontiguousarray(out.astype(np.float32))


def kernel(theta0, context, W1, b1, W2, b2, W3, b3, n_steps):
    return _build_and_run(
        np.asarray(theta0), np.asarray(context), W1, b1, W2, b2, W3, b3, n_steps
    )



# revision 20
# speedup vs baseline: 1.0614x; 1.0614x over previous
"""CCNF RK4 sampling kernel for 8 Trainium2 NeuronCores — v2.

Data-parallel across cores (2048 -> 256/core), and each core's batch is
split into TWO groups of 128 samples whose serial RK4 chains are
software-pipelined half-an-eval apart, so one group's L1 sigmoid/GLU
phase overlaps the other group's L2/L3 phase on the ACT/DVE engines.

The v1 kernel was latency-bound on the per-eval serial chain
(theta-MM -> 4x(sigma,GLU) -> L2 -> 4x(sigma,GLU) -> L3 -> RK4-STT ->
theta-MM', ~6.9us/eval).  v2 shortens the chain per group and hides the
rest with the second group:

  - whole-bank ops: sigma is ONE activation op per layer over a full
    [128, 4, 128] PSUM bank (4 chunks), GLU is ONE STT.  Bias made
    unnecessary: the time row t*W1[32] + b1 ride the theta-stationary
    ([34, 128]: theta rows + t row + ones row, maintained by gpsimd
    memsets on the idle Pool engine).
  - F-shortcut: tx = theta_s + c*k feeds L1 only through W1theta, so
    L1pre(e+1) = [ctx + theta_s + t] (pre-issued off-chain) +
    h2_e @ Fc where F = W3 @ W1[0:32] is precomputed host-side and
    applied as fp8 DoubleRow matmuls.  This removes L3->STT->theta-MM
    (two sem hops + a DVE op) from 3 of 4 eval boundaries.
  - RK4 combination in PSUM: acc += w_e * k_e via duplicate cheap L3
    DR matmuls with pre-scaled W3 variants; one STT per STEP updates
    the f32 theta state (thF), one ACT copy refreshes the bf16
    matmul-input copy.  (v1 spent 2 DVE STTs per eval here.)
  - fp8 scales: h2 is written scaled by s_h=1/4 (free in the GLU STT
    scalar) so Fc = F*c/s_h and w3 variants stay in fp8 normal range.

Numpy-probed accuracy of this exact quantization pipeline: 1.13e-2
(gate 2e-2).  Cost model: ~3.8us per eval-pair vs v1's 6.9us per eval.
"""

import os

import numpy as np
from ml_dtypes import bfloat16 as _bf16
from ml_dtypes import float8_e4m3 as _f8np

N_CORES = 8
G = 2          # pipelined sample groups per core
S_H = 0.25     # h2 scale carried in the GLU2 STT scalar


def _build_program(theta0, context, W1, b1, W2, b2, W3, b3, n_steps):
    import concourse.mybir as mybir
    import concourse.tile as tile
    from concourse import bacc

    f32 = mybir.dt.float32
    f32r = mybir.dt.float32r
    bf16 = mybir.dt.bfloat16
    f8 = mybir.dt.float8e4
    DR = mybir.MatmulPerfMode.DoubleRow
    ALU = mybir.AluOpType
    SIGMOID = mybir.ActivationFunctionType.Sigmoid

    B, D = theta0.shape          # 2048, 32
    C = context.shape[1]         # 128
    IN, H2 = W1.shape            # 161, 1024
    H = W2.shape[0]              # 512
    assert H2 == 2 * H and W2.shape[1] == 2 * H and W3.shape == (H, D)
    assert IN == D + 1 + C
    assert B % (N_CORES * G) == 0
    Bs = B // N_CORES            # 256
    Ng = Bs // G                 # 128
    steps = int(n_steps)
    dt = 1.0 / steps
    MJ = H // 128                # 4 column chunks per GLU half
    KCP = MJ // 2                # 2 DoubleRow pairs over the H contraction
    NEV = 4 * steps
    DT2 = D + 2                  # moving rows: theta(32) + t(1) + ones(1)

    b2f = np.asarray(b2, np.float32)
    b3f = np.asarray(b3, np.float32)
    b2nz = bool(np.any(b2f))
    b3nz = bool(np.any(b3f))

    # t value per eval (t = idx * dt/2)
    IOFF = (0, 1, 1, 2)
    TVAL = [(2 * (e // 4) + IOFF[e % 4]) * (dt / 2.0) for e in range(NEV + 1)]

    # ---- host-side layout prep (shared across cores) ----
    W1f = np.asarray(W1, np.float32)
    w1c_h = np.ascontiguousarray(W1f[D + 1:])                    # [128, 1024]
    w1tb_h = np.concatenate(
        [W1f[0:D + 1], np.asarray(b1, np.float32).reshape(1, 2 * H)], axis=0
    )                                                            # [34, 1024]
    w2_h = np.ascontiguousarray(
        np.asarray(W2, np.float32)
        .reshape(KCP, 2, 128, 2 * H).transpose(2, 0, 1, 3)
        .reshape(128, KCP * 2 * 2 * H)
    )

    W3f = np.asarray(W3, np.float32)

    def drpack(w, ncol):  # [H, ncol] -> [128, KCP, 2, ncol]
        return w.reshape(KCP, 2, 128, ncol).transpose(2, 0, 1, 3)

    # w3 variants scaled by w_e / s_h (w_e in {1, 2})
    w3v_h = np.ascontiguousarray(np.stack(
        [drpack(W3f * (1.0 / S_H), D), drpack(W3f * (2.0 / S_H), D)], axis=3
    ).reshape(128, KCP * 2 * 2 * D))                             # [128, P, pl, v, D]
    # F variants scaled by c_e / s_h (c_e in {dt/2, dt})
    F_h = W3f @ W1f[0:D]                                         # [512, 1024]
    fv_h = np.ascontiguousarray(np.stack(
        [drpack(F_h * (0.5 * dt / S_H), 2 * H), drpack(F_h * (dt / S_H), 2 * H)],
        axis=3,
    ).reshape(128, KCP * 2 * 2 * 2 * H))                         # [128, P, pl, v, 1024]

    # ---- build the bass program (same program on all 8 cores) ----
    nc = bacc.Bacc("TRN2", target_bir_lowering=False)

    d_ctxw = nc.dram_tensor("ctxw", [C, G * Ng + 2 * H], bf16, kind="ExternalInput")
    d_thw = nc.dram_tensor("thw", [DT2, G * Ng + 2 * H], bf16, kind="ExternalInput")
    d_thF = nc.dram_tensor("thF", [D, Bs], f32, kind="ExternalInput")
    d_w2 = nc.dram_tensor("w2", [128, KCP * 2 * 2 * H], f8, kind="ExternalInput")
    d_w3v = nc.dram_tensor("w3v", [128, KCP * 2 * 2 * D], f8, kind="ExternalInput")
    d_fv = nc.dram_tensor("fv", [128, KCP * 2 * 2 * 2 * H], f8, kind="ExternalInput")
    # bias fallbacks (all-zero in the reference problem): bias values ride as
    # single-row matmul stationaries against the ones row of the moving tile
    d_b2t = (nc.dram_tensor("b2t", [1, 2 * H], bf16, kind="ExternalInput")
             if b2nz else None)
    d_b3r = (nc.dram_tensor("b3r", [1, 2 * D + 2 * 2 * H], bf16, kind="ExternalInput")
             if b3nz else None)
    d_out = nc.dram_tensor("out", [D, Bs], f32, kind="ExternalOutput")

    DBG = bool(int(os.environ.get("KERNEL_DBG", "0")))
    d_dbg = {}
    if DBG:
        for nm, shp in (
            ("dbg_l1b_e0", [128, MJ * Ng]), ("dbg_sg1_e0", [128, MJ * Ng]),
            ("dbg_h1_e0", [128, MJ * Ng]), ("dbg_l2b_e0", [128, MJ * Ng]),
            ("dbg_h2_e0", [128, MJ * Ng]), ("dbg_acc_e0", [D, Ng]),
            ("dbg_l1b_e1", [128, MJ * Ng]), ("dbg_l1a_e1", [128, MJ * Ng]),
        ):
            d_dbg[nm] = nc.dram_tensor(nm, shp, f32, kind="ExternalOutput")

    PSB = int(os.environ.get("KERNEL_PSB", "6"))
    SGB = int(os.environ.get("KERNEL_SGB", "6"))
    HB = int(os.environ.get("KERNEL_HB", "6"))

    with tile.TileContext(nc) as tc:
        with (
            tc.tile_pool(name="const", bufs=1) as cpool,
            tc.tile_pool(name="psb", bufs=PSB, space="PSUM") as pspool,
            tc.tile_pool(name="pss", bufs=1, space="PSUM") as psspool,
            tc.tile_pool(name="sg", bufs=SGB) as sgpool,
            tc.tile_pool(name="h", bufs=HB) as hpool,
        ):
            tctxw = cpool.tile([C, G * Ng + 2 * H], bf16)
            tctx = [tctxw[:, g * Ng:(g + 1) * Ng] for g in range(G)]
            tthw = cpool.tile([DT2, G * Ng + 2 * H], bf16)
            tth = [tthw[:, g * Ng:(g + 1) * Ng] for g in range(G)]
            tthF = cpool.tile([D, G, Ng], f32)
            tw2 = cpool.tile([128, KCP, 2, 2 * H], f8)
            tw3v = cpool.tile([128, KCP, 2, 2, D], f8)
            tF = cpool.tile([128, KCP, 2, 2, 2 * H], f8)
            if b2nz:
                tb2 = cpool.tile([1, 2 * H], bf16)
            if b3nz:
                # cols: [b3 | 2*b3 | (dt/2)*b3@W1th | dt*b3@W1th]
                tb3 = cpool.tile([1, 2 * D + 2 * 2 * H], bf16)
            # one acc bank per group: PSUM start=True is bank-granular, so
            # the accumulators cannot share a bank with anything live
            psacc = [psspool.tile([D, Ng], f32, name=f"acc{g}") for g in range(G)]

            def w1c_col(half, j):
                base = G * Ng + half * H + j * 128
                return tctxw[:, base:base + 128]

            def w1tb_col(half, j):
                base = G * Ng + half * H + j * 128
                return tthw[:, base:base + 128]

            # startup DMAs: L1-critical tensors first, weights stream behind
            nc.sync.dma_start(tctxw[:], d_ctxw[:])
            nc.sync.dma_start(tthw[:], d_thw[:])
            nc.sync.dma_start(tthF[:], d_thF[:])
            for P in range(KCP):
                nc.sync.dma_start(tw2[:, P, :, :],
                                  d_w2[:, P * 2 * 2 * H:(P + 1) * 2 * 2 * H])
            nc.sync.dma_start(tw3v[:], d_w3v[:])
            nc.sync.dma_start(tF[:], d_fv[:])
            if b2nz:
                nc.sync.dma_start(tb2[:], d_b2t[:])
            if b3nz:
                nc.sync.dma_start(tb3[:], d_b3r[:])

            def mm(out_ap, lhsT, rhs, start, stop, pm=None):
                nc.tensor.matmul(out_ap, lhsT, rhs, start=start, stop=stop,
                                 perf_mode=pm)

            L1 = {}
            H1out = {}

            def issue_l1(g, close):
                """Pre-issue next eval's static L1 parts: ctx + (theta_s,
                t, b1) matmuls.  close=True ends the accumulation groups
                (step boundary, no F-term); else F matmuls close later."""
                bb = pspool.tile([128, MJ, Ng], f32, tag="bank", name=f"L1b{g}")
                ba = pspool.tile([128, MJ, Ng], f32, tag="bank", name=f"L1a{g}")
                for half, bank in ((1, bb), (0, ba)):
                    for j in range(MJ):
                        # start only on the bank's first MM: a second start
                        # re-marks the whole bank pending-zero
                        mm(bank[:, j, :], w1c_col(half, j), tctx[g][:],
                           start=(j == 0), stop=False)
                for half, bank in ((1, bb), (0, ba)):
                    for j in range(MJ):
                        mm(bank[:, j, :], w1tb_col(half, j), tth[g][:],
                           start=False, stop=close)
                return bb, ba

            def dbg_dump(nm, ap):
                if DBG and nm in d_dbg:
                    t = cpool.tile([ap.shape[0], int(np.prod(ap.shape[1:]))], f32,
                                   name=nm)
                    nc.scalar.copy(t[:], ap)
                    nc.sync.dma_start(d_dbg[nm][:], t[:])

            def phase1(g, e):
                # sigma1 over the whole b-bank, GLU1 -> fp8 pair tile
                bb, ba = L1[g]
                sg = sgpool.tile([128, MJ, Ng], bf16, tag="sg", name=f"sg1{g}")
                nc.scalar.activation(sg[:], bb[:, :, :], SIGMOID)
                h1 = hpool.tile([128, MJ, Ng], f8, tag="h1", name=f"h1{g}")
                nc.vector.scalar_tensor_tensor(h1[:], ba[:, :, :], 1.0, sg[:],
                                               ALU.mult, ALU.mult)
                H1out[g] = h1
                if DBG and g == 0 and e == 0:
                    dbg_dump("dbg_l1b_e0", bb[:, :, :])
                    dbg_dump("dbg_sg1_e0", sg[:])
                    dbg_dump("dbg_h1_e0", h1[:])
                if DBG and g == 0 and e == 1:
                    dbg_dump("dbg_l1b_e1", bb[:, :, :])
                    dbg_dump("dbg_l1a_e1", ba[:, :, :])

            def phase2(g, e):
                s, ei = divmod(e, 4)
                last = e == NEV - 1
                boundary = ei == 3
                h1 = H1out[g]
                bb2 = pspool.tile([128, MJ, Ng], f32, tag="bank", name=f"L2b{g}")
                ba2 = pspool.tile([128, MJ, Ng], f32, tag="bank", name=f"L2a{g}")
                for j in range(MJ):
                    for P in range(KCP):
                        mm(bb2[:, j, :], tw2[:, P, :, H + j * 128:H + (j + 1) * 128],
                           h1[:, 2 * P:2 * P + 2, :],
                           start=(j == 0 and P == 0),
                           stop=(P == KCP - 1 and not b2nz), pm=DR)
                if b2nz:  # fallback: bias via ones-row matmuls (b-half)
                    for j in range(MJ):
                        mm(bb2[:, j, :], tb2[:, H + j * 128:H + (j + 1) * 128],
                           tth[g][D + 1:D + 2, :], start=False, stop=True)
                sg2 = sgpool.tile([128, MJ, Ng], bf16, tag="sg", name=f"sg2{g}")
                nc.scalar.activation(sg2[:], bb2[:, :, :], SIGMOID)
                for j in range(MJ):
                    for P in range(KCP):
                        mm(ba2[:, j, :], tw2[:, P, :, j * 128:(j + 1) * 128],
                           h1[:, 2 * P:2 * P + 2, :],
                           start=(j == 0 and P == 0),
                           stop=(P == KCP - 1 and not b2nz), pm=DR)
                if b2nz:
                    for j in range(MJ):
                        mm(ba2[:, j, :], tb2[:, j * 128:(j + 1) * 128],
                           tth[g][D + 1:D + 2, :], start=False, stop=True)
                if not boundary:
                    # static L1 parts for e+1 overlap this eval's ACT/DVE tail
                    nc.gpsimd.memset(tth[g][D:D + 1, :], float(TVAL[e + 1]))
                    L1[g] = issue_l1(g, close=False)
                h2 = hpool.tile([128, MJ, Ng], f8, tag="h2", name=f"h2{g}")
                nc.vector.scalar_tensor_tensor(h2[:], ba2[:, :, :], S_H, sg2[:],
                                               ALU.mult, ALU.mult)
                # RK4 accumulator: acc += w_e * k_e (pre-scaled W3 variants)
                acc = psacc[g][:]
                v = 0 if ei in (0, 3) else 1
                for P in range(KCP):
                    mm(acc, tw3v[:, P, :, v, :], h2[:, 2 * P:2 * P + 2, :],
                       start=(ei == 0 and P == 0),
                       stop=(P == KCP - 1 and not b3nz), pm=DR)
                if b3nz:
                    boff = 0 if ei in (0, 3) else D
                    mm(acc, tb3[:, boff:boff + D],
                       tth[g][D + 1:D + 2, :], start=False, stop=True)
                if DBG and g == 0 and e == 0:
                    dbg_dump("dbg_l2b_e0", bb2[:, :, :])
                    dbg_dump("dbg_h2_e0", h2[:])
                    dbg_dump("dbg_acc_e0", acc)
                if not boundary:
                    # F-shortcut: theta-correction of the NEXT eval's L1 pre-
                    # activations directly from h2 (closes the L1 banks)
                    fv = 0 if ei < 2 else 1
                    bb, ba = L1[g]
                    for half, bank in ((1, bb), (0, ba)):
                        for j in range(MJ):
                            col = half * H + j * 128
                            for P in range(KCP):
                                mm(bank[:, j, :],
                                   tF[:, P, :, fv, col:col + 128],
                                   h2[:, 2 * P:2 * P + 2, :],
                                   start=False, stop=(P == KCP - 1), pm=DR)
                    if b3nz:
                        # c_e * (b3 @ W1theta) correction row, via ones row.
                        # NOTE: when b3nz, the F matmuls above must not stop
                        # the group; keep them stop=True and let this row use
                        # its own mini-group accumulate (start=False is fine).
                        for half, bank in ((1, bb), (0, ba)):
                            for j in range(MJ):
                                boff = 2 * D + fv * 2 * H + half * H + j * 128
                                mm(bank[:, j, :], tb3[:, boff:boff + 128],
                                   tth[g][D + 1:D + 2, :], start=False, stop=True)
                else:
                    # step boundary: fold acc into the f32 state, refresh the
                    # bf16 matmul copy, pre-issue e+1 (closed groups)
                    nc.vector.scalar_tensor_tensor(
                        tthF[:, g, :], acc, float(dt / 6.0), tthF[:, g, :],
                        ALU.mult, ALU.add)
                    if not last:
                        nc.scalar.copy(tth[g][0:D, :], tthF[:, g, :])
                        nc.gpsimd.memset(tth[g][D:D + 1, :], float(TVAL[e + 1]))
                        L1[g] = issue_l1(g, close=True)

            # ---- prologue: first-eval static parts for both groups ----
            L1[0] = issue_l1(0, close=True)
            L1[1] = issue_l1(1, close=True)

            # ---- half-slot walk: A leads, B lags by half an eval ----
            for hs in range(2 * NEV + 1):
                if hs % 2 == 0:
                    if hs < 2 * NEV:
                        phase1(0, hs // 2)
                    if hs >= 2:
                        phase2(1, hs // 2 - 1)
                else:
                    e = (hs - 1) // 2
                    phase1(1, e)
                    phase2(0, e)

            nc.sync.dma_start(d_out[:], tthF[:, :, :])

    # ---- per-core input maps ----
    w1c_b = w1c_h.astype(_bf16)
    w1tb_b = w1tb_h.astype(_bf16)
    w2_q = w2_h.astype(_f8np)
    w3v_q = w3v_h.astype(_f8np)
    fv_q = fv_h.astype(_f8np)
    if b2nz:
        b2t_h = np.ascontiguousarray(b2f.reshape(1, 2 * H)).astype(_bf16)
    if b3nz:
        b3w = (b3f @ W1f[0:D]).reshape(1, 2 * H)
        b3r_h = np.concatenate(
            [b3f.reshape(1, D), 2.0 * b3f.reshape(1, D),
             0.5 * dt * b3w, dt * b3w], axis=1).astype(_bf16)
    in_maps = []
    for c in range(N_CORES):
        sl = slice(c * Bs, (c + 1) * Bs)
        th_T = np.ascontiguousarray(np.asarray(theta0[sl], np.float32).T)  # [32,256]
        ctx_T = np.ascontiguousarray(np.asarray(context[sl], np.float32).T)  # [128,256]
        thg = []
        for g in range(G):
            t34 = np.zeros((DT2, Ng), np.float32)
            t34[0:D] = th_T[:, g * Ng:(g + 1) * Ng]
            t34[D] = 0.0          # t row (t=0 at start)
            t34[D + 1] = 1.0      # ones row
            thg.append(t34)
        thw = np.ascontiguousarray(np.concatenate(
            [np.concatenate(thg, axis=1).astype(_bf16), w1tb_b], axis=1))
        ctxw = np.ascontiguousarray(np.concatenate(
            [ctx_T.astype(_bf16), w1c_b], axis=1))
        m = {
            "ctxw": ctxw,
            "thw": thw,
            "thF": th_T,
            "w2": w2_q,
            "w3v": w3v_q,
            "fv": fv_q,
        }
        if b2nz:
            m["b2t"] = b2t_h
        if b3nz:
            m["b3r"] = b3r_h
        in_maps.append(m)

    return nc, in_maps


def _build_and_run(theta0, context, W1, b1, W2, b2, W3, b3, n_steps):
    from concourse.bass_utils import run_bass_kernel_spmd

    nc, in_maps = _build_program(theta0, context, W1, b1, W2, b2, W3, b3, n_steps)
    nc.finalize()
    res = run_bass_kernel_spmd(
        nc,
        in_maps,
        core_ids=list(range(N_CORES)),
        trace=bool(int(os.environ.get("KERNEL_TRACE", "0"))),
    )
    _build_and_run.last_results = res

    out = np.concatenate([r["out"].T for r in res.results], axis=0)
    return np.ascontiguousarray(out.astype(np.float32))


def kernel(theta0, context, W1, b1, W2, b2, W3, b3, n_steps):
    return _build_and_run(
        np.asarray(theta0), np.asarray(context), W1, b1, W2, b2, W3, b3, n_steps
    )


# revision 21
# speedup vs baseline: 1.3315x; 1.2544x over previous
"""CCNF RK4 sampling kernel for 8 Trainium2 NeuronCores — v2.

Data-parallel across cores (2048 -> 256/core), and each core's batch is
split into TWO groups of 128 samples whose serial RK4 chains are
software-pipelined half-an-eval apart, so one group's L1 sigmoid/GLU
phase overlaps the other group's L2/L3 phase on the ACT/DVE engines.

The v1 kernel was latency-bound on the per-eval serial chain
(theta-MM -> 4x(sigma,GLU) -> L2 -> 4x(sigma,GLU) -> L3 -> RK4-STT ->
theta-MM', ~6.9us/eval).  v2 shortens the chain per group and hides the
rest with the second group:

  - whole-bank ops: sigma is ONE activation op per layer over a full
    [128, 4, 128] PSUM bank (4 chunks), GLU is ONE STT.  Bias made
    unnecessary: the time row t*W1[32] + b1 ride the theta-stationary
    ([34, 128]: theta rows + t row + ones row, maintained by gpsimd
    memsets on the idle Pool engine).
  - F-shortcut: tx = theta_s + c*k feeds L1 only through W1theta, so
    L1pre(e+1) = [ctx + theta_s + t] (pre-issued off-chain) +
    h2_e @ Fc where F = W3 @ W1[0:32] is precomputed host-side and
    applied as fp8 DoubleRow matmuls.  This removes L3->STT->theta-MM
    (two sem hops + a DVE op) from 3 of 4 eval boundaries.
  - RK4 combination in PSUM: acc += w_e * k_e via duplicate cheap L3
    DR matmuls with pre-scaled W3 variants; one STT per STEP updates
    the f32 theta state (thF), one ACT copy refreshes the bf16
    matmul-input copy.  (v1 spent 2 DVE STTs per eval here.)
  - fp8 scales: h2 is written scaled by s_h=1/4 (free in the GLU STT
    scalar) so Fc = F*c/s_h and w3 variants stay in fp8 normal range.

Numpy-probed accuracy of this exact quantization pipeline: 1.13e-2
(gate 2e-2).  Cost model: ~3.8us per eval-pair vs v1's 6.9us per eval.
"""

import os

import numpy as np
from ml_dtypes import bfloat16 as _bf16
from ml_dtypes import float8_e4m3 as _f8np

N_CORES = 8
G = 2          # pipelined sample groups per core
S_H = 0.25     # h2 scale carried in the GLU2 STT scalar


def _build_program(theta0, context, W1, b1, W2, b2, W3, b3, n_steps):
    import concourse.mybir as mybir
    import concourse.tile as tile
    from concourse import bacc

    f32 = mybir.dt.float32
    f32r = mybir.dt.float32r
    bf16 = mybir.dt.bfloat16
    f8 = mybir.dt.float8e4
    DR = mybir.MatmulPerfMode.DoubleRow
    ALU = mybir.AluOpType
    SIGMOID = mybir.ActivationFunctionType.Sigmoid

    B, D = theta0.shape          # 2048, 32
    C = context.shape[1]         # 128
    IN, H2 = W1.shape            # 161, 1024
    H = W2.shape[0]              # 512
    assert H2 == 2 * H and W2.shape[1] == 2 * H and W3.shape == (H, D)
    assert IN == D + 1 + C
    assert B % (N_CORES * G) == 0
    Bs = B // N_CORES            # 256
    Ng = Bs // G                 # 128
    steps = int(n_steps)
    dt = 1.0 / steps
    MJ = H // 128                # 4 column chunks per GLU half
    KCP = MJ // 2                # 2 DoubleRow pairs over the H contraction
    NEV = 4 * steps
    DT2 = D + 2                  # moving rows: theta(32) + t(1) + ones(1)

    b2f = np.asarray(b2, np.float32)
    b3f = np.asarray(b3, np.float32)
    b2nz = bool(np.any(b2f))
    b3nz = bool(np.any(b3f))

    # t value per eval (t = idx * dt/2)
    IOFF = (0, 1, 1, 2)
    TVAL = [(2 * (e // 4) + IOFF[e % 4]) * (dt / 2.0) for e in range(NEV + 1)]

    # ---- host-side layout prep (shared across cores) ----
    W1f = np.asarray(W1, np.float32)
    w1c_h = np.ascontiguousarray(W1f[D + 1:])                    # [128, 1024]
    w1tb_h = np.concatenate(
        [W1f[0:D + 1], np.asarray(b1, np.float32).reshape(1, 2 * H)], axis=0
    )                                                            # [34, 1024]
    w2_h = np.ascontiguousarray(
        np.asarray(W2, np.float32)
        .reshape(KCP, 2, 128, 2 * H).transpose(2, 0, 1, 3)
        .reshape(128, KCP * 2 * 2 * H)
    )

    W3f = np.asarray(W3, np.float32)

    def drpack(w, ncol):  # [H, ncol] -> [128, KCP, 2, ncol]
        return w.reshape(KCP, 2, 128, ncol).transpose(2, 0, 1, 3)

    # w3 variants scaled by w_e / s_h (w_e in {1, 2})
    w3v_h = np.ascontiguousarray(np.stack(
        [drpack(W3f * (1.0 / S_H), D), drpack(W3f * (2.0 / S_H), D)], axis=3
    ).reshape(128, KCP * 2 * 2 * D))                             # [128, P, pl, v, D]
    # F variants scaled by c_e / s_h (c_e in {dt/2, dt})
    F_h = W3f @ W1f[0:D]                                         # [512, 1024]
    fv_h = np.ascontiguousarray(np.stack(
        [drpack(F_h * (0.5 * dt / S_H), 2 * H), drpack(F_h * (dt / S_H), 2 * H)],
        axis=3,
    ).reshape(128, KCP * 2 * 2 * 2 * H))                         # [128, P, pl, v, 1024]

    # ---- build the bass program (same program on all 8 cores) ----
    nc = bacc.Bacc("TRN2", target_bir_lowering=False)

    d_ctxw = nc.dram_tensor("ctxw", [C, G * Ng + 2 * H], bf16, kind="ExternalInput")
    d_thw = nc.dram_tensor("thw", [DT2, G * Ng + 2 * H], bf16, kind="ExternalInput")
    d_thF = nc.dram_tensor("thF", [D, Bs], f32, kind="ExternalInput")
    d_w2 = nc.dram_tensor("w2", [128, KCP * 2 * 2 * H], f8, kind="ExternalInput")
    d_w3v = nc.dram_tensor("w3v", [128, KCP * 2 * 2 * D], f8, kind="ExternalInput")
    d_fv = nc.dram_tensor("fv", [128, KCP * 2 * 2 * 2 * H], f8, kind="ExternalInput")
    # bias fallbacks (all-zero in the reference problem): bias values ride as
    # single-row matmul stationaries against the ones row of the moving tile
    d_b2t = (nc.dram_tensor("b2t", [1, 2 * H], bf16, kind="ExternalInput")
             if b2nz else None)
    d_b3r = (nc.dram_tensor("b3r", [1, 2 * D + 2 * 2 * H], bf16, kind="ExternalInput")
             if b3nz else None)
    d_out = nc.dram_tensor("out", [D, Bs], f32, kind="ExternalOutput")

    DBG = bool(int(os.environ.get("KERNEL_DBG", "0")))
    d_dbg = {}
    if DBG:
        for nm, shp in (
            ("dbg_l1b_e0", [128, MJ * Ng]), ("dbg_sg1_e0", [128, MJ * Ng]),
            ("dbg_h1_e0", [128, MJ * Ng]), ("dbg_l2b_e0", [128, MJ * Ng]),
            ("dbg_h2_e0", [128, MJ * Ng]), ("dbg_acc_e0", [D, Ng]),
            ("dbg_l1b_e1", [128, MJ * Ng]), ("dbg_l1a_e1", [128, MJ * Ng]),
        ):
            d_dbg[nm] = nc.dram_tensor(nm, shp, f32, kind="ExternalOutput")

    PSB = int(os.environ.get("KERNEL_PSB", "6"))
    SGB = int(os.environ.get("KERNEL_SGB", "6"))
    HB = int(os.environ.get("KERNEL_HB", "6"))

    with tile.TileContext(nc) as tc:
        with (
            tc.tile_pool(name="const", bufs=1) as cpool,
            tc.tile_pool(name="psb", bufs=PSB, space="PSUM") as pspool,
            tc.tile_pool(name="pss", bufs=1, space="PSUM") as psspool,
            tc.tile_pool(name="sg", bufs=SGB) as sgpool,
            tc.tile_pool(name="h", bufs=HB) as hpool,
        ):
            tctxw = cpool.tile([C, G * Ng + 2 * H], bf16)
            tctx = [tctxw[:, g * Ng:(g + 1) * Ng] for g in range(G)]
            tthw = cpool.tile([DT2, G * Ng + 2 * H], bf16)
            tth = [tthw[:, g * Ng:(g + 1) * Ng] for g in range(G)]
            tthF = cpool.tile([D, G, Ng], f32)
            tw2 = cpool.tile([128, KCP, 2, 2 * H], f8)
            tw3v = cpool.tile([128, KCP, 2, 2, D], f8)
            tF = cpool.tile([128, KCP, 2, 2, 2 * H], f8)
            if b2nz:
                tb2 = cpool.tile([1, 2 * H], bf16)
            if b3nz:
                # cols: [b3 | 2*b3 | (dt/2)*b3@W1th | dt*b3@W1th]
                tb3 = cpool.tile([1, 2 * D + 2 * 2 * H], bf16)
            # one acc bank per group: PSUM start=True is bank-granular, so
            # the accumulators cannot share a bank with anything live
            psacc = [psspool.tile([D, Ng], f32, name=f"acc{g}") for g in range(G)]

            def w1c_col(half, j):
                base = G * Ng + half * H + j * 128
                return tctxw[:, base:base + 128]

            def w1tb_col(half, j):
                base = G * Ng + half * H + j * 128
                return tthw[:, base:base + 128]

            # startup DMAs: L1-critical tensors first, weights stream behind
            nc.sync.dma_start(tctxw[:], d_ctxw[:])
            nc.sync.dma_start(tthw[:], d_thw[:])
            nc.sync.dma_start(tthF[:], d_thF[:])
            for P in range(KCP):
                nc.sync.dma_start(tw2[:, P, :, :],
                                  d_w2[:, P * 2 * 2 * H:(P + 1) * 2 * 2 * H])
            nc.sync.dma_start(tw3v[:], d_w3v[:])
            nc.sync.dma_start(tF[:], d_fv[:])
            if b2nz:
                nc.sync.dma_start(tb2[:], d_b2t[:])
            if b3nz:
                nc.sync.dma_start(tb3[:], d_b3r[:])

            def mm(out_ap, lhsT, rhs, start, stop, pm=None):
                nc.tensor.matmul(out_ap, lhsT, rhs, start=start, stop=stop,
                                 perf_mode=pm)

            L1 = {}
            H1out = {}

            def issue_l1(g, close):
                """Pre-issue next eval's static L1 parts: ctx + (theta_s,
                t, b1) matmuls.  close=True ends the accumulation groups
                (step boundary, no F-term); else F matmuls close later."""
                bb = pspool.tile([128, MJ, Ng], f32, tag="bank", name=f"L1b{g}")
                ba = pspool.tile([128, MJ, Ng], f32, tag="bank", name=f"L1a{g}")
                for half, bank in ((1, bb), (0, ba)):
                    for j in range(MJ):
                        # start only on the bank's first MM: a second start
                        # re-marks the whole bank pending-zero
                        mm(bank[:, j, :], w1c_col(half, j), tctx[g][:],
                           start=(j == 0), stop=False)
                for half, bank in ((1, bb), (0, ba)):
                    for j in range(MJ):
                        mm(bank[:, j, :], w1tb_col(half, j), tth[g][:],
                           start=False, stop=close)
                return bb, ba

            def dbg_dump(nm, ap):
                if DBG and nm in d_dbg:
                    t = cpool.tile([ap.shape[0], int(np.prod(ap.shape[1:]))], f32,
                                   name=nm)
                    nc.scalar.copy(t[:], ap)
                    nc.sync.dma_start(d_dbg[nm][:], t[:])

            def phase1(g, e):
                # sigma1 over the whole b-bank, GLU1 -> fp8 pair tile
                bb, ba = L1[g]
                sg = sgpool.tile([128, MJ, Ng], bf16, tag="sg", name=f"sg1{g}")
                nc.scalar.activation(sg[:], bb[:, :, :], SIGMOID)
                h1 = hpool.tile([128, MJ, Ng], f8, tag="h1", name=f"h1{g}")
                nc.vector.scalar_tensor_tensor(h1[:], ba[:, :, :], 1.0, sg[:],
                                               ALU.mult, ALU.mult)
                H1out[g] = h1
                if DBG and g == 0 and e == 0:
                    dbg_dump("dbg_l1b_e0", bb[:, :, :])
                    dbg_dump("dbg_sg1_e0", sg[:])
                    dbg_dump("dbg_h1_e0", h1[:])
                if DBG and g == 0 and e == 1:
                    dbg_dump("dbg_l1b_e1", bb[:, :, :])
                    dbg_dump("dbg_l1a_e1", ba[:, :, :])

            L2banks = {}

            def phase2L(g, e):
                # L2 matmuls + sigma2 (L2a runs behind sigma2 on the PE)
                h1 = H1out[g]
                bb2 = pspool.tile([128, MJ, Ng], f32, tag="bank", name=f"L2b{g}")
                ba2 = pspool.tile([128, MJ, Ng], f32, tag="bank", name=f"L2a{g}")
                for j in range(MJ):
                    for P in range(KCP):
                        mm(bb2[:, j, :], tw2[:, P, :, H + j * 128:H + (j + 1) * 128],
                           h1[:, 2 * P:2 * P + 2, :],
                           start=(j == 0 and P == 0),
                           stop=(P == KCP - 1 and not b2nz), pm=DR)
                if b2nz:  # fallback: bias via ones-row matmuls (b-half)
                    for j in range(MJ):
                        mm(bb2[:, j, :], tb2[:, H + j * 128:H + (j + 1) * 128],
                           tth[g][D + 1:D + 2, :], start=False, stop=True)
                sg2 = sgpool.tile([128, MJ, Ng], bf16, tag="sg", name=f"sg2{g}")
                nc.scalar.activation(sg2[:], bb2[:, :, :], SIGMOID)
                for j in range(MJ):
                    for P in range(KCP):
                        mm(ba2[:, j, :], tw2[:, P, :, j * 128:(j + 1) * 128],
                           h1[:, 2 * P:2 * P + 2, :],
                           start=(j == 0 and P == 0),
                           stop=(P == KCP - 1 and not b2nz), pm=DR)
                if b2nz:
                    for j in range(MJ):
                        mm(ba2[:, j, :], tb2[:, j * 128:(j + 1) * 128],
                           tth[g][D + 1:D + 2, :], start=False, stop=True)
                L2banks[g] = (bb2, ba2, sg2)

            def pre_issue(g, e):
                # static L1 parts of eval e (ctx + theta_s + t + b1) — no data
                # deps beyond tth/tctx, so these fill PE dependency-wait gaps.
                # Skipped for e%4==0 (issued in phase2T after the state copy).
                nc.gpsimd.memset(tth[g][D:D + 1, :], float(TVAL[e]))
                L1[g] = issue_l1(g, close=False)

            def phase2T(g, e):
                # GLU2 -> acc matmuls -> F-shortcut (or boundary state update)
                s, ei = divmod(e, 4)
                last = e == NEV - 1
                boundary = ei == 3
                bb2, ba2, sg2 = L2banks[g]
                h2 = hpool.tile([128, MJ, Ng], f8, tag="h2", name=f"h2{g}")
                nc.vector.scalar_tensor_tensor(h2[:], ba2[:, :, :], S_H, sg2[:],
                                               ALU.mult, ALU.mult)
                # RK4 accumulator: acc += w_e * k_e (pre-scaled W3 variants)
                acc = psacc[g][:]
                v = 0 if ei in (0, 3) else 1
                for P in range(KCP):
                    mm(acc, tw3v[:, P, :, v, :], h2[:, 2 * P:2 * P + 2, :],
                       start=(ei == 0 and P == 0),
                       stop=(P == KCP - 1 and not b3nz), pm=DR)
                if b3nz:
                    boff = 0 if ei in (0, 3) else D
                    mm(acc, tb3[:, boff:boff + D],
                       tth[g][D + 1:D + 2, :], start=False, stop=True)
                if DBG and g == 0 and e == 0:
                    dbg_dump("dbg_l2b_e0", bb2[:, :, :])
                    dbg_dump("dbg_h2_e0", h2[:])
                    dbg_dump("dbg_acc_e0", acc)
                if not boundary:
                    # F-shortcut: theta-correction of the NEXT eval's L1 pre-
                    # activations directly from h2 (closes the L1 banks);
                    # b-bank first so sigma1 unblocks before GLU1 needs a-bank
                    fv = 0 if ei < 2 else 1
                    bb, ba = L1[g]
                    for half, bank in ((1, bb), (0, ba)):
                        for j in range(MJ):
                            col = half * H + j * 128
                            for P in range(KCP):
                                mm(bank[:, j, :],
                                   tF[:, P, :, fv, col:col + 128],
                                   h2[:, 2 * P:2 * P + 2, :],
                                   start=False,
                                   stop=(P == KCP - 1 and not b3nz), pm=DR)
                    if b3nz:
                        # c_e * (b3 @ W1theta) correction row via the ones row
                        for half, bank in ((1, bb), (0, ba)):
                            for j in range(MJ):
                                boff = 2 * D + fv * 2 * H + half * H + j * 128
                                mm(bank[:, j, :], tb3[:, boff:boff + 128],
                                   tth[g][D + 1:D + 2, :], start=False, stop=True)
                else:
                    # step boundary: fold acc into the f32 state, refresh the
                    # bf16 matmul copy, pre-issue e+1 (closed groups)
                    nc.vector.scalar_tensor_tensor(
                        tthF[:, g, :], acc, float(dt / 6.0), tthF[:, g, :],
                        ALU.mult, ALU.add)
                    if not last:
                        nc.scalar.copy(tth[g][0:D, :], tthF[:, g, :])
                        nc.gpsimd.memset(tth[g][D:D + 1, :], float(TVAL[e + 1]))
                        L1[g] = issue_l1(g, close=True)

            # ---- prologue: first-eval static parts for both groups ----
            L1[0] = issue_l1(0, close=True)
            L1[1] = issue_l1(1, close=True)

            # ---- slot walk: B lags A by a quarter period.  Per slot the
            # engine streams are ACT [s1A, s2B, s2A, s1B], DVE [g1A, g2B,
            # g2A, g1B], PE [L2B, preB', L2A, accB+FB, preA', accA+FA] so
            # neither group's chain waits on the other's long segments. ----
            for e in range(NEV):
                phase1(0, e)
                if e > 0:
                    phase2L(1, e - 1)
                    if (e - 1) % 4 != 3:
                        pre_issue(1, e)
                phase2L(0, e)
                if e > 0:
                    phase2T(1, e - 1)
                if e % 4 != 3:
                    pre_issue(0, e + 1)
                phase2T(0, e)
                phase1(1, e)
            phase2L(1, NEV - 1)
            phase2T(1, NEV - 1)

            nc.sync.dma_start(d_out[:], tthF[:, :, :])

    # ---- per-core input maps ----
    w1c_b = w1c_h.astype(_bf16)
    w1tb_b = w1tb_h.astype(_bf16)
    w2_q = w2_h.astype(_f8np)
    w3v_q = w3v_h.astype(_f8np)
    fv_q = fv_h.astype(_f8np)
    if b2nz:
        b2t_h = np.ascontiguousarray(b2f.reshape(1, 2 * H)).astype(_bf16)
    if b3nz:
        b3w = (b3f @ W1f[0:D]).reshape(1, 2 * H)
        b3r_h = np.concatenate(
            [b3f.reshape(1, D), 2.0 * b3f.reshape(1, D),
             0.5 * dt * b3w, dt * b3w], axis=1).astype(_bf16)
    in_maps = []
    for c in range(N_CORES):
        sl = slice(c * Bs, (c + 1) * Bs)
        th_T = np.ascontiguousarray(np.asarray(theta0[sl], np.float32).T)  # [32,256]
        ctx_T = np.ascontiguousarray(np.asarray(context[sl], np.float32).T)  # [128,256]
        thg = []
        for g in range(G):
            t34 = np.zeros((DT2, Ng), np.float32)
            t34[0:D] = th_T[:, g * Ng:(g + 1) * Ng]
            t34[D] = 0.0          # t row (t=0 at start)
            t34[D + 1] = 1.0      # ones row
            thg.append(t34)
        thw = np.ascontiguousarray(np.concatenate(
            [np.concatenate(thg, axis=1).astype(_bf16), w1tb_b], axis=1))
        ctxw = np.ascontiguousarray(np.concatenate(
            [ctx_T.astype(_bf16), w1c_b], axis=1))
        m = {
            "ctxw": ctxw,
            "thw": thw,
            "thF": th_T,
            "w2": w2_q,
            "w3v": w3v_q,
            "fv": fv_q,
        }
        if b2nz:
            m["b2t"] = b2t_h
        if b3nz:
            m["b3r"] = b3r_h
        in_maps.append(m)

    return nc, in_maps


def _build_and_run(theta0, context, W1, b1, W2, b2, W3, b3, n_steps):
    from concourse.bass_utils import run_bass_kernel_spmd

    nc, in_maps = _build_program(theta0, context, W1, b1, W2, b2, W3, b3, n_steps)
    nc.finalize()
    res = run_bass_kernel_spmd(
        nc,
        in_maps,
        core_ids=list(range(N_CORES)),
        trace=bool(int(os.environ.get("KERNEL_TRACE", "0"))),
    )
    _build_and_run.last_results = res

    out = np.concatenate([r["out"].T for r in res.results], axis=0)
    return np.ascontiguousarray(out.astype(np.float32))


def kernel(theta0, context, W1, b1, W2, b2, W3, b3, n_steps):
    return _build_and_run(
        np.asarray(theta0), np.asarray(context), W1, b1, W2, b2, W3, b3, n_steps
    )


# revision 22
# speedup vs baseline: 1.3937x; 1.0468x over previous
"""CCNF RK4 sampling kernel for 8 Trainium2 NeuronCores — v2.

Data-parallel across cores (2048 -> 256/core), and each core's batch is
split into TWO groups of 128 samples whose serial RK4 chains are
software-pipelined half-an-eval apart, so one group's L1 sigmoid/GLU
phase overlaps the other group's L2/L3 phase on the ACT/DVE engines.

The v1 kernel was latency-bound on the per-eval serial chain
(theta-MM -> 4x(sigma,GLU) -> L2 -> 4x(sigma,GLU) -> L3 -> RK4-STT ->
theta-MM', ~6.9us/eval).  v2 shortens the chain per group and hides the
rest with the second group:

  - whole-bank ops: sigma is ONE activation op per layer over a full
    [128, 4, 128] PSUM bank (4 chunks), GLU is ONE STT.  Bias made
    unnecessary: the time row t*W1[32] + b1 ride the theta-stationary
    ([34, 128]: theta rows + t row + ones row, maintained by gpsimd
    memsets on the idle Pool engine).
  - F-shortcut: tx = theta_s + c*k feeds L1 only through W1theta, so
    L1pre(e+1) = [ctx + theta_s + t] (pre-issued off-chain) +
    h2_e @ Fc where F = W3 @ W1[0:32] is precomputed host-side and
    applied as fp8 DoubleRow matmuls.  This removes L3->STT->theta-MM
    (two sem hops + a DVE op) from 3 of 4 eval boundaries.
  - RK4 combination in PSUM: acc += w_e * k_e via duplicate cheap L3
    DR matmuls with pre-scaled W3 variants; one STT per STEP updates
    the f32 theta state (thF), one ACT copy refreshes the bf16
    matmul-input copy.  (v1 spent 2 DVE STTs per eval here.)
  - fp8 scales: h2 is written scaled by s_h=1/4 (free in the GLU STT
    scalar) so Fc = F*c/s_h and w3 variants stay in fp8 normal range.

Numpy-probed accuracy of this exact quantization pipeline: 1.13e-2
(gate 2e-2).  Cost model: ~3.8us per eval-pair vs v1's 6.9us per eval.
"""

import os

import numpy as np
from ml_dtypes import bfloat16 as _bf16
from ml_dtypes import float8_e4m3 as _f8np

N_CORES = 8
G = 2          # pipelined sample groups per core
S_H = 0.25     # h2 scale carried in the GLU2 STT scalar


def _build_program(theta0, context, W1, b1, W2, b2, W3, b3, n_steps):
    import concourse.mybir as mybir
    import concourse.tile as tile
    from concourse import bacc

    f32 = mybir.dt.float32
    f32r = mybir.dt.float32r
    bf16 = mybir.dt.bfloat16
    f8 = mybir.dt.float8e4
    DR = mybir.MatmulPerfMode.DoubleRow
    ALU = mybir.AluOpType
    SIGMOID = mybir.ActivationFunctionType.Sigmoid

    B, D = theta0.shape          # 2048, 32
    C = context.shape[1]         # 128
    IN, H2 = W1.shape            # 161, 1024
    H = W2.shape[0]              # 512
    assert H2 == 2 * H and W2.shape[1] == 2 * H and W3.shape == (H, D)
    assert IN == D + 1 + C
    assert B % (N_CORES * G) == 0
    Bs = B // N_CORES            # 256
    Ng = Bs // G                 # 128
    steps = int(n_steps)
    dt = 1.0 / steps
    MJ = H // 128                # 4 column chunks per GLU half
    KCP = MJ // 2                # 2 DoubleRow pairs over the H contraction
    NEV = 4 * steps
    DT2 = D + 2                  # moving rows: theta(32) + t(1) + ones(1)

    b2f = np.asarray(b2, np.float32)
    b3f = np.asarray(b3, np.float32)
    b2nz = bool(np.any(b2f))
    b3nz = bool(np.any(b3f))

    # t value per eval (t = idx * dt/2)
    IOFF = (0, 1, 1, 2)
    TVAL = [(2 * (e // 4) + IOFF[e % 4]) * (dt / 2.0) for e in range(NEV + 1)]

    # ---- host-side layout prep (shared across cores) ----
    W1f = np.asarray(W1, np.float32)
    w1c_h = np.ascontiguousarray(W1f[D + 1:])                    # [128, 1024]
    w1tb_h = np.concatenate(
        [W1f[0:D + 1], np.asarray(b1, np.float32).reshape(1, 2 * H)], axis=0
    )                                                            # [34, 1024]
    w2_h = np.ascontiguousarray(
        np.asarray(W2, np.float32)
        .reshape(KCP, 2, 128, 2 * H).transpose(2, 0, 1, 3)
        .reshape(128, KCP * 2 * 2 * H)
    )

    W3f = np.asarray(W3, np.float32)

    def drpack(w, ncol):  # [H, ncol] -> [128, KCP, 2, ncol]
        return w.reshape(KCP, 2, 128, ncol).transpose(2, 0, 1, 3)

    # w3 variants scaled by w_e / s_h (w_e in {1, 2})
    w3v_h = np.ascontiguousarray(np.stack(
        [drpack(W3f * (1.0 / S_H), D), drpack(W3f * (2.0 / S_H), D)], axis=3
    ).reshape(128, KCP * 2 * 2 * D))                             # [128, P, pl, v, D]
    # F variants scaled by c_e / s_h (c_e in {dt/2, dt})
    F_h = W3f @ W1f[0:D]                                         # [512, 1024]
    fv_h = np.ascontiguousarray(np.stack(
        [drpack(F_h * (0.5 * dt / S_H), 2 * H), drpack(F_h * (dt / S_H), 2 * H)],
        axis=3,
    ).reshape(128, KCP * 2 * 2 * 2 * H))                         # [128, P, pl, v, 1024]

    # ---- build the bass program (same program on all 8 cores) ----
    nc = bacc.Bacc("TRN2", target_bir_lowering=False)

    d_ctxw = nc.dram_tensor("ctxw", [C, G * Ng + 2 * H], bf16, kind="ExternalInput")
    d_thw = nc.dram_tensor("thw", [DT2, G * Ng + 2 * H], bf16, kind="ExternalInput")
    d_thF = nc.dram_tensor("thF", [D, Bs], f32, kind="ExternalInput")
    d_w2 = nc.dram_tensor("w2", [128, KCP * 2 * 2 * H], f8, kind="ExternalInput")
    d_w3v = nc.dram_tensor("w3v", [128, KCP * 2 * 2 * D], f8, kind="ExternalInput")
    d_fv = nc.dram_tensor("fv", [128, KCP * 2 * 2 * 2 * H], f8, kind="ExternalInput")
    # bias fallbacks (all-zero in the reference problem): bias values ride as
    # single-row matmul stationaries against the ones row of the moving tile
    d_b2t = (nc.dram_tensor("b2t", [1, 2 * H], bf16, kind="ExternalInput")
             if b2nz else None)
    d_b3r = (nc.dram_tensor("b3r", [1, 2 * D + 2 * 2 * H], bf16, kind="ExternalInput")
             if b3nz else None)
    d_out = nc.dram_tensor("out", [D, Bs], f32, kind="ExternalOutput")

    DBG = bool(int(os.environ.get("KERNEL_DBG", "0")))
    d_dbg = {}
    if DBG:
        for nm, shp in (
            ("dbg_l1b_e0", [128, MJ * Ng]), ("dbg_sg1_e0", [128, MJ * Ng]),
            ("dbg_h1_e0", [128, MJ * Ng]), ("dbg_l2b_e0", [128, MJ * Ng]),
            ("dbg_h2_e0", [128, MJ * Ng]), ("dbg_acc_e0", [D, Ng]),
            ("dbg_l1b_e1", [128, MJ * Ng]), ("dbg_l1a_e1", [128, MJ * Ng]),
        ):
            d_dbg[nm] = nc.dram_tensor(nm, shp, f32, kind="ExternalOutput")

    PSB = int(os.environ.get("KERNEL_PSB", "6"))
    SGB = int(os.environ.get("KERNEL_SGB", "6"))
    HB = int(os.environ.get("KERNEL_HB", "6"))

    with tile.TileContext(nc) as tc:
        with (
            tc.tile_pool(name="const", bufs=1) as cpool,
            tc.tile_pool(name="psb", bufs=PSB, space="PSUM") as pspool,
            tc.tile_pool(name="pss", bufs=1, space="PSUM") as psspool,
            tc.tile_pool(name="sg", bufs=SGB) as sgpool,
            tc.tile_pool(name="h", bufs=HB) as hpool,
        ):
            tctxw = cpool.tile([C, G * Ng + 2 * H], bf16)
            tctx = [tctxw[:, g * Ng:(g + 1) * Ng] for g in range(G)]
            tthw = cpool.tile([DT2, G * Ng + 2 * H], bf16)
            tth = [tthw[:, g * Ng:(g + 1) * Ng] for g in range(G)]
            tthF = cpool.tile([D, G, Ng], f32)
            tw2 = cpool.tile([128, KCP, 2, 2 * H], f8)
            tw3v = cpool.tile([128, KCP, 2, 2, D], f8)
            tF = cpool.tile([128, KCP, 2, 2, 2 * H], f8)
            if b2nz:
                tb2 = cpool.tile([1, 2 * H], bf16)
            if b3nz:
                # cols: [b3 | 2*b3 | (dt/2)*b3@W1th | dt*b3@W1th]
                tb3 = cpool.tile([1, 2 * D + 2 * 2 * H], bf16)
            # one acc bank per group: PSUM start=True is bank-granular, so
            # the accumulators cannot share a bank with anything live
            psacc = [psspool.tile([D, Ng], f32, name=f"acc{g}") for g in range(G)]

            def w1c_col(half, j):
                base = G * Ng + half * H + j * 128
                return tctxw[:, base:base + 128]

            def w1tb_col(half, j):
                base = G * Ng + half * H + j * 128
                return tthw[:, base:base + 128]

            # startup DMAs: L1-critical tensors first, weights stream behind
            nc.sync.dma_start(tctxw[:], d_ctxw[:])
            nc.sync.dma_start(tthw[:], d_thw[:])
            nc.sync.dma_start(tthF[:], d_thF[:])
            for P in range(KCP):
                nc.sync.dma_start(tw2[:, P, :, :],
                                  d_w2[:, P * 2 * 2 * H:(P + 1) * 2 * 2 * H])
            nc.sync.dma_start(tw3v[:], d_w3v[:])
            nc.sync.dma_start(tF[:], d_fv[:])
            if b2nz:
                nc.sync.dma_start(tb2[:], d_b2t[:])
            if b3nz:
                nc.sync.dma_start(tb3[:], d_b3r[:])

            def mm(out_ap, lhsT, rhs, start, stop, pm=None):
                nc.tensor.matmul(out_ap, lhsT, rhs, start=start, stop=stop,
                                 perf_mode=pm)

            L1 = {}
            H1out = {}

            def issue_l1(g, close):
                """Pre-issue next eval's static L1 parts: ctx + (theta_s,
                t, b1) matmuls.  close=True ends the accumulation groups
                (step boundary, no F-term); else F matmuls close later."""
                bb = pspool.tile([128, MJ, Ng], f32, tag="bank", name=f"L1b{g}")
                ba = pspool.tile([128, MJ, Ng], f32, tag="bank", name=f"L1a{g}")
                for half, bank in ((1, bb), (0, ba)):
                    for j in range(MJ):
                        # start only on the bank's first MM: a second start
                        # re-marks the whole bank pending-zero
                        mm(bank[:, j, :], w1c_col(half, j), tctx[g][:],
                           start=(j == 0), stop=False)
                for half, bank in ((1, bb), (0, ba)):
                    for j in range(MJ):
                        mm(bank[:, j, :], w1tb_col(half, j), tth[g][:],
                           start=False, stop=close)
                return bb, ba

            def dbg_dump(nm, ap):
                if DBG and nm in d_dbg:
                    t = cpool.tile([ap.shape[0], int(np.prod(ap.shape[1:]))], f32,
                                   name=nm)
                    nc.scalar.copy(t[:], ap)
                    nc.sync.dma_start(d_dbg[nm][:], t[:])

            def phase1(g, e):
                # sigma1 over the whole b-bank, GLU1 -> fp8 pair tile
                bb, ba = L1[g]
                sg = sgpool.tile([128, MJ, Ng], bf16, tag="sg", name=f"sg1{g}")
                nc.scalar.activation(sg[:], bb[:, :, :], SIGMOID)
                h1 = hpool.tile([128, MJ, Ng], f8, tag="h1", name=f"h1{g}")
                nc.vector.scalar_tensor_tensor(h1[:], ba[:, :, :], 1.0, sg[:],
                                               ALU.mult, ALU.mult)
                H1out[g] = h1
                if DBG and g == 0 and e == 0:
                    dbg_dump("dbg_l1b_e0", bb[:, :, :])
                    dbg_dump("dbg_sg1_e0", sg[:])
                    dbg_dump("dbg_h1_e0", h1[:])
                if DBG and g == 0 and e == 1:
                    dbg_dump("dbg_l1b_e1", bb[:, :, :])
                    dbg_dump("dbg_l1a_e1", ba[:, :, :])

            L2banks = {}

            def phase2L(g, e):
                # L2 matmuls + sigma2 (L2a runs behind sigma2 on the PE)
                h1 = H1out[g]
                bb2 = pspool.tile([128, MJ, Ng], f32, tag="bank", name=f"L2b{g}")
                ba2 = pspool.tile([128, MJ, Ng], f32, tag="bank", name=f"L2a{g}")
                for j in range(MJ):
                    for P in range(KCP):
                        mm(bb2[:, j, :], tw2[:, P, :, H + j * 128:H + (j + 1) * 128],
                           h1[:, 2 * P:2 * P + 2, :],
                           start=(j == 0 and P == 0),
                           stop=(P == KCP - 1 and not b2nz), pm=DR)
                if b2nz:  # fallback: bias via ones-row matmuls (b-half)
                    for j in range(MJ):
                        mm(bb2[:, j, :], tb2[:, H + j * 128:H + (j + 1) * 128],
                           tth[g][D + 1:D + 2, :], start=False, stop=True)
                sg2 = sgpool.tile([128, MJ, Ng], bf16, tag="sg", name=f"sg2{g}")
                nc.scalar.activation(sg2[:], bb2[:, :, :], SIGMOID)
                for j in range(MJ):
                    for P in range(KCP):
                        mm(ba2[:, j, :], tw2[:, P, :, j * 128:(j + 1) * 128],
                           h1[:, 2 * P:2 * P + 2, :],
                           start=(j == 0 and P == 0),
                           stop=(P == KCP - 1 and not b2nz), pm=DR)
                if b2nz:
                    for j in range(MJ):
                        mm(ba2[:, j, :], tb2[:, j * 128:(j + 1) * 128],
                           tth[g][D + 1:D + 2, :], start=False, stop=True)
                L2banks[g] = (bb2, ba2, sg2)

            def pre_issue(g, e):
                # static L1 parts of eval e (ctx + theta_s + t + b1) — no data
                # deps beyond tth/tctx, so these fill PE dependency-wait gaps.
                # Skipped for e%4==0 (issued in phase2T after the state copy).
                nc.gpsimd.memset(tth[g][D:D + 1, :], float(TVAL[e]))
                L1[g] = issue_l1(g, close=False)

            def phase2T(g, e):
                # GLU2 -> acc matmuls -> F-shortcut (or boundary state update)
                s, ei = divmod(e, 4)
                last = e == NEV - 1
                boundary = ei == 3
                bb2, ba2, sg2 = L2banks[g]
                h2 = hpool.tile([128, MJ, Ng], f8, tag="h2", name=f"h2{g}")
                nc.vector.scalar_tensor_tensor(h2[:], ba2[:, :, :], S_H, sg2[:],
                                               ALU.mult, ALU.mult)
                # RK4 accumulator: acc += w_e * k_e (pre-scaled W3 variants)
                acc = psacc[g][:]
                v = 0 if ei in (0, 3) else 1
                for P in range(KCP):
                    mm(acc, tw3v[:, P, :, v, :], h2[:, 2 * P:2 * P + 2, :],
                       start=(ei == 0 and P == 0),
                       stop=(P == KCP - 1 and not b3nz), pm=DR)
                if b3nz:
                    boff = 0 if ei in (0, 3) else D
                    mm(acc, tb3[:, boff:boff + D],
                       tth[g][D + 1:D + 2, :], start=False, stop=True)
                if DBG and g == 0 and e == 0:
                    dbg_dump("dbg_l2b_e0", bb2[:, :, :])
                    dbg_dump("dbg_h2_e0", h2[:])
                    dbg_dump("dbg_acc_e0", acc)
                if not boundary:
                    # F-shortcut: theta-correction of the NEXT eval's L1 pre-
                    # activations directly from h2 (closes the L1 banks);
                    # b-bank first so sigma1 unblocks before GLU1 needs a-bank
                    fv = 0 if ei < 2 else 1
                    bb, ba = L1[g]
                    for half, bank in ((1, bb), (0, ba)):
                        for j in range(MJ):
                            col = half * H + j * 128
                            for P in range(KCP):
                                mm(bank[:, j, :],
                                   tF[:, P, :, fv, col:col + 128],
                                   h2[:, 2 * P:2 * P + 2, :],
                                   start=False,
                                   stop=(P == KCP - 1 and not b3nz), pm=DR)
                    if b3nz:
                        # c_e * (b3 @ W1theta) correction row via the ones row
                        for half, bank in ((1, bb), (0, ba)):
                            for j in range(MJ):
                                boff = 2 * D + fv * 2 * H + half * H + j * 128
                                mm(bank[:, j, :], tb3[:, boff:boff + 128],
                                   tth[g][D + 1:D + 2, :], start=False, stop=True)
                elif last:
                    nc.vector.scalar_tensor_tensor(
                        tthF[:, g, :], acc, float(dt / 6.0), tthF[:, g, :],
                        ALU.mult, ALU.add)
                else:
                    # step boundary.  Critical chain: acc -> bf16 theta tile
                    # -> theta matmuls -> sigma1(e+1).  The f32 state update
                    # (same inputs) runs behind it, off-chain.
                    nc.vector.scalar_tensor_tensor(
                        tth[g][0:D, :], acc, float(dt / 6.0), tthF[:, g, :],
                        ALU.mult, ALU.add)
                    nc.gpsimd.memset(tth[g][D:D + 1, :], float(TVAL[e + 1]))
                    L1[g] = issue_l1(g, close=True)
                    nc.vector.scalar_tensor_tensor(
                        tthF[:, g, :], acc, float(dt / 6.0), tthF[:, g, :],
                        ALU.mult, ALU.add)

            # ---- prologue: first-eval static parts for both groups ----
            L1[0] = issue_l1(0, close=True)
            L1[1] = issue_l1(1, close=True)

            # ---- slot walk: B lags A by a quarter period.  Per slot the
            # engine streams are ACT [s1A, s2B, s2A, s1B], DVE [g1A, g2B,
            # g2A, g1B], PE [L2B, preB', L2A, accB+FB, preA', accA+FA] so
            # neither group's chain waits on the other's long segments. ----
            for e in range(NEV):
                phase1(0, e)
                if e > 0:
                    phase2L(1, e - 1)
                    if (e - 1) % 4 != 3:
                        pre_issue(1, e)
                phase2L(0, e)
                if e > 0:
                    phase2T(1, e - 1)
                if e % 4 != 3:
                    pre_issue(0, e + 1)
                phase2T(0, e)
                phase1(1, e)
            phase2L(1, NEV - 1)
            phase2T(1, NEV - 1)

            nc.sync.dma_start(d_out[:], tthF[:, :, :])

    # ---- per-core input maps ----
    w1c_b = w1c_h.astype(_bf16)
    w1tb_b = w1tb_h.astype(_bf16)
    w2_q = w2_h.astype(_f8np)
    w3v_q = w3v_h.astype(_f8np)
    fv_q = fv_h.astype(_f8np)
    if b2nz:
        b2t_h = np.ascontiguousarray(b2f.reshape(1, 2 * H)).astype(_bf16)
    if b3nz:
        b3w = (b3f @ W1f[0:D]).reshape(1, 2 * H)
        b3r_h = np.concatenate(
            [b3f.reshape(1, D), 2.0 * b3f.reshape(1, D),
             0.5 * dt * b3w, dt * b3w], axis=1).astype(_bf16)
    in_maps = []
    for c in range(N_CORES):
        sl = slice(c * Bs, (c + 1) * Bs)
        th_T = np.ascontiguousarray(np.asarray(theta0[sl], np.float32).T)  # [32,256]
        ctx_T = np.ascontiguousarray(np.asarray(context[sl], np.float32).T)  # [128,256]
        thg = []
        for g in range(G):
            t34 = np.zeros((DT2, Ng), np.float32)
            t34[0:D] = th_T[:, g * Ng:(g + 1) * Ng]
            t34[D] = 0.0          # t row (t=0 at start)
            t34[D + 1] = 1.0      # ones row
            thg.append(t34)
        thw = np.ascontiguousarray(np.concatenate(
            [np.concatenate(thg, axis=1).astype(_bf16), w1tb_b], axis=1))
        ctxw = np.ascontiguousarray(np.concatenate(
            [ctx_T.astype(_bf16), w1c_b], axis=1))
        m = {
            "ctxw": ctxw,
            "thw": thw,
            "thF": th_T,
            "w2": w2_q,
            "w3v": w3v_q,
            "fv": fv_q,
        }
        if b2nz:
            m["b2t"] = b2t_h
        if b3nz:
            m["b3r"] = b3r_h
        in_maps.append(m)

    return nc, in_maps


def _build_and_run(theta0, context, W1, b1, W2, b2, W3, b3, n_steps):
    from concourse.bass_utils import run_bass_kernel_spmd

    nc, in_maps = _build_program(theta0, context, W1, b1, W2, b2, W3, b3, n_steps)
    nc.finalize()
    res = run_bass_kernel_spmd(
        nc,
        in_maps,
        core_ids=list(range(N_CORES)),
        trace=bool(int(os.environ.get("KERNEL_TRACE", "0"))),
    )
    _build_and_run.last_results = res

    out = np.concatenate([r["out"].T for r in res.results], axis=0)
    return np.ascontiguousarray(out.astype(np.float32))


def kernel(theta0, context, W1, b1, W2, b2, W3, b3, n_steps):
    return _build_and_run(
        np.asarray(theta0), np.asarray(context), W1, b1, W2, b2, W3, b3, n_steps
    )


# revision 24
# speedup vs baseline: 1.4053x; 1.0083x over previous
"""CCNF RK4 sampling kernel for 8 Trainium2 NeuronCores — v2.

Data-parallel across cores (2048 -> 256/core), and each core's batch is
split into TWO groups of 128 samples whose serial RK4 chains are
software-pipelined half-an-eval apart, so one group's L1 sigmoid/GLU
phase overlaps the other group's L2/L3 phase on the ACT/DVE engines.

The v1 kernel was latency-bound on the per-eval serial chain
(theta-MM -> 4x(sigma,GLU) -> L2 -> 4x(sigma,GLU) -> L3 -> RK4-STT ->
theta-MM', ~6.9us/eval).  v2 shortens the chain per group and hides the
rest with the second group:

  - whole-bank ops: sigma is ONE activation op per layer over a full
    [128, 4, 128] PSUM bank (4 chunks), GLU is ONE STT.  Bias made
    unnecessary: the time row t*W1[32] + b1 ride the theta-stationary
    ([34, 128]: theta rows + t row + ones row, maintained by gpsimd
    memsets on the idle Pool engine).
  - F-shortcut: tx = theta_s + c*k feeds L1 only through W1theta, so
    L1pre(e+1) = [ctx + theta_s + t] (pre-issued off-chain) +
    h2_e @ Fc where F = W3 @ W1[0:32] is precomputed host-side and
    applied as fp8 DoubleRow matmuls.  This removes L3->STT->theta-MM
    (two sem hops + a DVE op) from 3 of 4 eval boundaries.
  - RK4 combination in PSUM: acc += w_e * k_e via duplicate cheap L3
    DR matmuls with pre-scaled W3 variants; one STT per STEP updates
    the f32 theta state (thF), one ACT copy refreshes the bf16
    matmul-input copy.  (v1 spent 2 DVE STTs per eval here.)
  - fp8 scales: h2 is written scaled by s_h=1/4 (free in the GLU STT
    scalar) so Fc = F*c/s_h and w3 variants stay in fp8 normal range.

Numpy-probed accuracy of this exact quantization pipeline: 1.13e-2
(gate 2e-2).  Cost model: ~3.8us per eval-pair vs v1's 6.9us per eval.
"""

import os

import numpy as np
from ml_dtypes import bfloat16 as _bf16
from ml_dtypes import float8_e4m3 as _f8np

N_CORES = 8
G = 2          # pipelined sample groups per core
OP_LABELS = {}  # instruction name -> human label (for the trace analyzer)
S_H = 0.25     # h2 scale carried in the GLU2 STT scalar


def _build_program(theta0, context, W1, b1, W2, b2, W3, b3, n_steps):
    import concourse.mybir as mybir
    import concourse.tile as tile
    from concourse import bacc

    f32 = mybir.dt.float32
    f32r = mybir.dt.float32r
    bf16 = mybir.dt.bfloat16
    f8 = mybir.dt.float8e4
    DR = mybir.MatmulPerfMode.DoubleRow
    ALU = mybir.AluOpType
    SIGMOID = mybir.ActivationFunctionType.Sigmoid

    B, D = theta0.shape          # 2048, 32
    C = context.shape[1]         # 128
    IN, H2 = W1.shape            # 161, 1024
    H = W2.shape[0]              # 512
    assert H2 == 2 * H and W2.shape[1] == 2 * H and W3.shape == (H, D)
    assert IN == D + 1 + C
    assert B % (N_CORES * G) == 0
    Bs = B // N_CORES            # 256
    Ng = Bs // G                 # 128
    steps = int(n_steps)
    dt = 1.0 / steps
    MJ = H // 128                # 4 column chunks per GLU half
    KCP = MJ // 2                # 2 DoubleRow pairs over the H contraction
    NEV = 4 * steps
    DT2 = D + 2                  # moving rows: theta(32) + t(1) + ones(1)

    b2f = np.asarray(b2, np.float32)
    b3f = np.asarray(b3, np.float32)
    b2nz = bool(np.any(b2f))
    b3nz = bool(np.any(b3f))

    # t value per eval (t = idx * dt/2)
    IOFF = (0, 1, 1, 2)
    TVAL = [(2 * (e // 4) + IOFF[e % 4]) * (dt / 2.0) for e in range(NEV + 1)]

    # ---- host-side layout prep (shared across cores) ----
    W1f = np.asarray(W1, np.float32)
    w1c_h = np.ascontiguousarray(W1f[D + 1:])                    # [128, 1024]
    w1tb_h = np.concatenate(
        [W1f[0:D + 1], np.asarray(b1, np.float32).reshape(1, 2 * H)], axis=0
    )                                                            # [34, 1024]
    w2_h = np.ascontiguousarray(
        np.asarray(W2, np.float32)
        .reshape(KCP, 2, 128, 2 * H).transpose(2, 0, 1, 3)
        .reshape(128, KCP * 2 * 2 * H)
    )

    W3f = np.asarray(W3, np.float32)

    def drpack(w, ncol):  # [H, ncol] -> [128, KCP, 2, ncol]
        return w.reshape(KCP, 2, 128, ncol).transpose(2, 0, 1, 3)

    # w3 variants scaled by w_e / s_h (w_e in {1, 2})
    w3v_h = np.ascontiguousarray(np.stack(
        [drpack(W3f * (1.0 / S_H), D), drpack(W3f * (2.0 / S_H), D)], axis=3
    ).reshape(128, KCP * 2 * 2 * D))                             # [128, P, pl, v, D]
    # F variants scaled by c_e / s_h (c_e in {dt/2, dt})
    F_h = W3f @ W1f[0:D]                                         # [512, 1024]
    fv_h = np.ascontiguousarray(np.stack(
        [drpack(F_h * (0.5 * dt / S_H), 2 * H), drpack(F_h * (dt / S_H), 2 * H)],
        axis=3,
    ).reshape(128, KCP * 2 * 2 * 2 * H))                         # [128, P, pl, v, 1024]

    # ---- build the bass program (same program on all 8 cores) ----
    nc = bacc.Bacc("TRN2", target_bir_lowering=False)

    d_ctxw = nc.dram_tensor("ctxw", [C, G * Ng + 2 * H], bf16, kind="ExternalInput")
    d_thw = nc.dram_tensor("thw", [DT2, G * Ng + 2 * H], bf16, kind="ExternalInput")
    d_thF = nc.dram_tensor("thF", [D, Bs], f32, kind="ExternalInput")
    d_w2 = nc.dram_tensor("w2", [128, KCP * 2 * 2 * H], f8, kind="ExternalInput")
    d_w3v = nc.dram_tensor("w3v", [128, KCP * 2 * 2 * D], f8, kind="ExternalInput")
    d_fv = nc.dram_tensor("fv", [128, KCP * 2 * 2 * 2 * H], f8, kind="ExternalInput")
    # bias fallbacks (all-zero in the reference problem): bias values ride as
    # single-row matmul stationaries against the ones row of the moving tile
    d_b2t = (nc.dram_tensor("b2t", [1, 2 * H], bf16, kind="ExternalInput")
             if b2nz else None)
    d_b3r = (nc.dram_tensor("b3r", [1, 2 * D + 2 * 2 * H], bf16, kind="ExternalInput")
             if b3nz else None)
    d_out = nc.dram_tensor("out", [D, Bs], f32, kind="ExternalOutput")

    DBG = bool(int(os.environ.get("KERNEL_DBG", "0")))
    d_dbg = {}
    if DBG:
        for nm, shp in (
            ("dbg_l1b_e0", [128, MJ * Ng]), ("dbg_sg1_e0", [128, MJ * Ng]),
            ("dbg_h1_e0", [128, MJ * Ng]), ("dbg_l2b_e0", [128, MJ * Ng]),
            ("dbg_h2_e0", [128, MJ * Ng]), ("dbg_acc_e0", [D, Ng]),
            ("dbg_l1b_e1", [128, MJ * Ng]), ("dbg_l1a_e1", [128, MJ * Ng]),
        ):
            d_dbg[nm] = nc.dram_tensor(nm, shp, f32, kind="ExternalOutput")

    PSB = int(os.environ.get("KERNEL_PSB", "6"))
    SGB = int(os.environ.get("KERNEL_SGB", "6"))
    HB = int(os.environ.get("KERNEL_HB", "6"))

    with tile.TileContext(nc) as tc:
        with (
            tc.tile_pool(name="const", bufs=1) as cpool,
            tc.tile_pool(name="psb", bufs=PSB, space="PSUM") as pspool,
            tc.tile_pool(name="pss", bufs=1, space="PSUM") as psspool,
            tc.tile_pool(name="sg", bufs=SGB) as sgpool,
            tc.tile_pool(name="h", bufs=HB) as hpool,
        ):
            tctxw = cpool.tile([C, G * Ng + 2 * H], bf16)
            tctx = [tctxw[:, g * Ng:(g + 1) * Ng] for g in range(G)]
            tthw = cpool.tile([DT2, G * Ng + 2 * H], bf16)
            tth = [tthw[:, g * Ng:(g + 1) * Ng] for g in range(G)]
            tthF = cpool.tile([D, G, Ng], f32)
            tw2 = cpool.tile([128, KCP, 2, 2 * H], f8)
            tw3v = cpool.tile([128, KCP, 2, 2, D], f8)
            tF = cpool.tile([128, KCP, 2, 2, 2 * H], f8)
            if b2nz:
                tb2 = cpool.tile([1, 2 * H], bf16)
            if b3nz:
                # cols: [b3 | 2*b3 | (dt/2)*b3@W1th | dt*b3@W1th]
                tb3 = cpool.tile([1, 2 * D + 2 * 2 * H], bf16)
            # one acc bank per group: PSUM start=True is bank-granular, so
            # the accumulators cannot share a bank with anything live
            psacc = [psspool.tile([D, Ng], f32, name=f"acc{g}") for g in range(G)]

            def w1c_col(half, j):
                base = G * Ng + half * H + j * 128
                return tctxw[:, base:base + 128]

            def w1tb_col(half, j):
                base = G * Ng + half * H + j * 128
                return tthw[:, base:base + 128]

            # startup DMAs: L1-critical tensors first, weights stream behind
            nc.sync.dma_start(tctxw[:], d_ctxw[:])
            nc.sync.dma_start(tthw[:], d_thw[:])
            nc.sync.dma_start(tthF[:], d_thF[:])
            for P in range(KCP):
                nc.sync.dma_start(tw2[:, P, :, :],
                                  d_w2[:, P * 2 * 2 * H:(P + 1) * 2 * 2 * H])
            nc.sync.dma_start(tw3v[:], d_w3v[:])
            nc.sync.dma_start(tF[:], d_fv[:])
            if b2nz:
                nc.sync.dma_start(tb2[:], d_b2t[:])
            if b3nz:
                nc.sync.dma_start(tb3[:], d_b3r[:])

            CUR = ["?"]

            def lab(inst, name):
                try:
                    OP_LABELS[inst.name] = name
                except Exception:
                    pass
                return inst

            def mm(out_ap, lhsT, rhs, start, stop, pm=None):
                lab(nc.tensor.matmul(out_ap, lhsT, rhs, start=start, stop=stop,
                                     perf_mode=pm), CUR[0])

            L1 = {}
            H1out = {}

            def issue_l1(g, close):
                """Pre-issue next eval's static L1 parts: ctx + (theta_s,
                t, b1) matmuls.  close=True ends the accumulation groups
                (step boundary, no F-term); else F matmuls close later."""
                bb = pspool.tile([128, MJ, Ng], f32, tag="bank", name=f"L1b{g}")
                ba = pspool.tile([128, MJ, Ng], f32, tag="bank", name=f"L1a{g}")
                CUR[0] = f"ctxMM.{g}"
                for half, bank in ((1, bb), (0, ba)):
                    for j in range(MJ):
                        # start only on the bank's first MM: a second start
                        # re-marks the whole bank pending-zero
                        mm(bank[:, j, :], w1c_col(half, j), tctx[g][:],
                           start=(j == 0), stop=False)
                CUR[0] = f"thMM.{g}"
                for half, bank in ((1, bb), (0, ba)):
                    for j in range(MJ):
                        mm(bank[:, j, :], w1tb_col(half, j), tth[g][:],
                           start=False, stop=close)
                return bb, ba

            def dbg_dump(nm, ap):
                if DBG and nm in d_dbg:
                    t = cpool.tile([ap.shape[0], int(np.prod(ap.shape[1:]))], f32,
                                   name=nm)
                    nc.scalar.copy(t[:], ap)
                    nc.sync.dma_start(d_dbg[nm][:], t[:])

            def phase1(g, e):
                # sigma1 over the whole b-bank, GLU1 -> fp8 pair tile
                bb, ba = L1[g]
                sg = sgpool.tile([128, MJ, Ng], bf16, tag="sg", name=f"sg1{g}")
                lab(nc.scalar.activation(sg[:], bb[:, :, :], SIGMOID), f"sig1.{g}.{e}")
                h1 = hpool.tile([128, MJ, Ng], f8, tag="h1", name=f"h1{g}")
                lab(nc.vector.scalar_tensor_tensor(h1[:], ba[:, :, :], 1.0, sg[:],
                                                   ALU.mult, ALU.mult), f"glu1.{g}.{e}")
                H1out[g] = h1
                if DBG and g == 0 and e == 0:
                    dbg_dump("dbg_l1b_e0", bb[:, :, :])
                    dbg_dump("dbg_sg1_e0", sg[:])
                    dbg_dump("dbg_h1_e0", h1[:])
                if DBG and g == 0 and e == 1:
                    dbg_dump("dbg_l1b_e1", bb[:, :, :])
                    dbg_dump("dbg_l1a_e1", ba[:, :, :])

            L2banks = {}

            def phase2L(g, e):
                # L2 matmuls + sigma2 (L2a runs behind sigma2 on the PE)
                h1 = H1out[g]
                CUR[0] = f"L2MM.{g}.{e}"
                bb2 = pspool.tile([128, MJ, Ng], f32, tag="bank", name=f"L2b{g}")
                ba2 = pspool.tile([128, MJ, Ng], f32, tag="bank", name=f"L2a{g}")
                for j in range(MJ):
                    for P in range(KCP):
                        mm(bb2[:, j, :], tw2[:, P, :, H + j * 128:H + (j + 1) * 128],
                           h1[:, 2 * P:2 * P + 2, :],
                           start=(j == 0 and P == 0),
                           stop=(P == KCP - 1 and not b2nz), pm=DR)
                if b2nz:  # fallback: bias via ones-row matmuls (b-half)
                    for j in range(MJ):
                        mm(bb2[:, j, :], tb2[:, H + j * 128:H + (j + 1) * 128],
                           tth[g][D + 1:D + 2, :], start=False, stop=True)
                sg2 = sgpool.tile([128, MJ, Ng], bf16, tag="sg", name=f"sg2{g}")
                lab(nc.scalar.activation(sg2[:], bb2[:, :, :], SIGMOID), f"sig2.{g}.{e}")
                for j in range(MJ):
                    for P in range(KCP):
                        mm(ba2[:, j, :], tw2[:, P, :, j * 128:(j + 1) * 128],
                           h1[:, 2 * P:2 * P + 2, :],
                           start=(j == 0 and P == 0),
                           stop=(P == KCP - 1 and not b2nz), pm=DR)
                if b2nz:
                    for j in range(MJ):
                        mm(ba2[:, j, :], tb2[:, j * 128:(j + 1) * 128],
                           tth[g][D + 1:D + 2, :], start=False, stop=True)
                L2banks[g] = (bb2, ba2, sg2)

            def pre_issue(g, e):
                # static L1 parts of eval e (ctx + theta_s + t + b1) — no data
                # deps beyond tth/tctx, so these fill PE dependency-wait gaps.
                # Skipped for e%4==0 (issued in phase2T after the state copy).
                nc.gpsimd.memset(tth[g][D:D + 1, :], float(TVAL[e]))
                L1[g] = issue_l1(g, close=False)

            def phase2T(g, e):
                # GLU2 -> acc matmuls -> F-shortcut (or boundary state update)
                s, ei = divmod(e, 4)
                last = e == NEV - 1
                boundary = ei == 3
                bb2, ba2, sg2 = L2banks[g]
                h2 = hpool.tile([128, MJ, Ng], f8, tag="h2", name=f"h2{g}")
                lab(nc.vector.scalar_tensor_tensor(h2[:], ba2[:, :, :], S_H, sg2[:],
                                                   ALU.mult, ALU.mult), f"glu2.{g}.{e}")
                acc = psacc[g][:]
                v = 0 if ei in (0, 3) else 1

                def acc_mms():
                    # RK4 accumulator: acc += w_e * k_e (pre-scaled W3)
                    CUR[0] = f"accMM.{g}.{e}"
                    for P in range(KCP):
                        mm(acc, tw3v[:, P, :, v, :], h2[:, 2 * P:2 * P + 2, :],
                           start=(ei == 0 and P == 0),
                           stop=(P == KCP - 1 and not b3nz), pm=DR)
                    if b3nz:
                        boff = 0 if ei in (0, 3) else D
                        mm(acc, tb3[:, boff:boff + D],
                           tth[g][D + 1:D + 2, :], start=False, stop=True)

                if boundary:
                    acc_mms()   # thSTT depends on acc: keep it first
                if DBG and g == 0 and e == 0:
                    dbg_dump("dbg_l2b_e0", bb2[:, :, :])
                    dbg_dump("dbg_h2_e0", h2[:])
                if not boundary:
                    # F-shortcut: theta-correction of the NEXT eval's L1 pre-
                    # activations directly from h2 (closes the L1 banks);
                    # b-bank first so sigma1 unblocks before GLU1 needs a-bank
                    CUR[0] = f"FMM.{g}.{e}"
                    fv = 0 if ei < 2 else 1
                    bb, ba = L1[g]
                    for half, bank in ((1, bb), (0, ba)):
                        for j in range(MJ):
                            col = half * H + j * 128
                            for P in range(KCP):
                                mm(bank[:, j, :],
                                   tF[:, P, :, fv, col:col + 128],
                                   h2[:, 2 * P:2 * P + 2, :],
                                   start=False,
                                   stop=(P == KCP - 1 and not b3nz), pm=DR)
                    if b3nz:
                        # c_e * (b3 @ W1theta) correction row via the ones row
                        for half, bank in ((1, bb), (0, ba)):
                            for j in range(MJ):
                                boff = 2 * D + fv * 2 * H + half * H + j * 128
                                mm(bank[:, j, :], tb3[:, boff:boff + 128],
                                   tth[g][D + 1:D + 2, :], start=False, stop=True)
                    acc_mms()  # off the critical chain on fast boundaries
                elif last:
                    nc.vector.scalar_tensor_tensor(
                        tthF[:, g, :], acc, float(dt / 6.0), tthF[:, g, :],
                        ALU.mult, ALU.add)
                else:
                    # step boundary.  Critical chain: acc -> bf16 theta tile
                    # -> theta matmuls -> sigma1(e+1).  The f32 state update
                    # (same inputs) runs behind it, off-chain.
                    lab(nc.vector.scalar_tensor_tensor(
                        tth[g][0:D, :], acc, float(dt / 6.0), tthF[:, g, :],
                        ALU.mult, ALU.add), f"thSTT.{g}.{e}")
                    nc.gpsimd.memset(tth[g][D:D + 1, :], float(TVAL[e + 1]))
                    L1[g] = issue_l1(g, close=True)
                    nc.vector.scalar_tensor_tensor(
                        tthF[:, g, :], acc, float(dt / 6.0), tthF[:, g, :],
                        ALU.mult, ALU.add)

            # ---- prologue: first-eval static parts for both groups ----
            L1[0] = issue_l1(0, close=True)
            L1[1] = issue_l1(1, close=True)

            # ---- slot walk: B lags A by a quarter period.  Per slot the
            # engine streams are ACT [s1A, s2B, s2A, s1B], DVE [g1A, g2B,
            # g2A, g1B], PE [L2B, preB', L2A, accB+FB, preA', accA+FA] so
            # neither group's chain waits on the other's long segments. ----
            for e in range(NEV):
                phase1(0, e)
                if e > 0:
                    phase2L(1, e - 1)
                    if (e - 1) % 4 != 3:
                        pre_issue(1, e)
                phase2L(0, e)
                if e > 0:
                    phase2T(1, e - 1)
                if e % 4 != 3:
                    pre_issue(0, e + 1)
                phase2T(0, e)
                phase1(1, e)
            phase2L(1, NEV - 1)
            phase2T(1, NEV - 1)

            nc.sync.dma_start(d_out[:], tthF[:, :, :])

    # ---- per-core input maps ----
    w1c_b = w1c_h.astype(_bf16)
    w1tb_b = w1tb_h.astype(_bf16)
    w2_q = w2_h.astype(_f8np)
    w3v_q = w3v_h.astype(_f8np)
    fv_q = fv_h.astype(_f8np)
    if b2nz:
        b2t_h = np.ascontiguousarray(b2f.reshape(1, 2 * H)).astype(_bf16)
    if b3nz:
        b3w = (b3f @ W1f[0:D]).reshape(1, 2 * H)
        b3r_h = np.concatenate(
            [b3f.reshape(1, D), 2.0 * b3f.reshape(1, D),
             0.5 * dt * b3w, dt * b3w], axis=1).astype(_bf16)
    in_maps = []
    for c in range(N_CORES):
        sl = slice(c * Bs, (c + 1) * Bs)
        th_T = np.ascontiguousarray(np.asarray(theta0[sl], np.float32).T)  # [32,256]
        ctx_T = np.ascontiguousarray(np.asarray(context[sl], np.float32).T)  # [128,256]
        thg = []
        for g in range(G):
            t34 = np.zeros((DT2, Ng), np.float32)
            t34[0:D] = th_T[:, g * Ng:(g + 1) * Ng]
            t34[D] = 0.0          # t row (t=0 at start)
            t34[D + 1] = 1.0      # ones row
            thg.append(t34)
        thw = np.ascontiguousarray(np.concatenate(
            [np.concatenate(thg, axis=1).astype(_bf16), w1tb_b], axis=1))
        ctxw = np.ascontiguousarray(np.concatenate(
            [ctx_T.astype(_bf16), w1c_b], axis=1))
        m = {
            "ctxw": ctxw,
            "thw": thw,
            "thF": th_T,
            "w2": w2_q,
            "w3v": w3v_q,
            "fv": fv_q,
        }
        if b2nz:
            m["b2t"] = b2t_h
        if b3nz:
            m["b3r"] = b3r_h
        in_maps.append(m)

    return nc, in_maps


def _build_and_run(theta0, context, W1, b1, W2, b2, W3, b3, n_steps):
    from concourse.bass_utils import run_bass_kernel_spmd

    nc, in_maps = _build_program(theta0, context, W1, b1, W2, b2, W3, b3, n_steps)
    nc.finalize()
    res = run_bass_kernel_spmd(
        nc,
        in_maps,
        core_ids=list(range(N_CORES)),
        trace=bool(int(os.environ.get("KERNEL_TRACE", "0"))),
    )
    _build_and_run.last_results = res

    out = np.concatenate([r["out"].T for r in res.results], axis=0)
    return np.ascontiguousarray(out.astype(np.float32))


def kernel(theta0, context, W1, b1, W2, b2, W3, b3, n_steps):
    return _build_and_run(
        np.asarray(theta0), np.asarray(context), W1, b1, W2, b2, W3, b3, n_steps
    )


# revision 25
# speedup vs baseline: 1.4174x; 1.0087x over previous
"""CCNF RK4 sampling kernel for 8 Trainium2 NeuronCores — v2.

Data-parallel across cores (2048 -> 256/core), and each core's batch is
split into TWO groups of 128 samples whose serial RK4 chains are
software-pipelined half-an-eval apart, so one group's L1 sigmoid/GLU
phase overlaps the other group's L2/L3 phase on the ACT/DVE engines.

The v1 kernel was latency-bound on the per-eval serial chain
(theta-MM -> 4x(sigma,GLU) -> L2 -> 4x(sigma,GLU) -> L3 -> RK4-STT ->
theta-MM', ~6.9us/eval).  v2 shortens the chain per group and hides the
rest with the second group:

  - whole-bank ops: sigma is ONE activation op per layer over a full
    [128, 4, 128] PSUM bank (4 chunks), GLU is ONE STT.  Bias made
    unnecessary: the time row t*W1[32] + b1 ride the theta-stationary
    ([34, 128]: theta rows + t row + ones row, maintained by gpsimd
    memsets on the idle Pool engine).
  - F-shortcut: tx = theta_s + c*k feeds L1 only through W1theta, so
    L1pre(e+1) = [ctx + theta_s + t] (pre-issued off-chain) +
    h2_e @ Fc where F = W3 @ W1[0:32] is precomputed host-side and
    applied as fp8 DoubleRow matmuls.  This removes L3->STT->theta-MM
    (two sem hops + a DVE op) from 3 of 4 eval boundaries.
  - RK4 combination in PSUM: acc += w_e * k_e via duplicate cheap L3
    DR matmuls with pre-scaled W3 variants; one STT per STEP updates
    the f32 theta state (thF), one ACT copy refreshes the bf16
    matmul-input copy.  (v1 spent 2 DVE STTs per eval here.)
  - fp8 scales: h2 is written scaled by s_h=1/4 (free in the GLU STT
    scalar) so Fc = F*c/s_h and w3 variants stay in fp8 normal range.

Numpy-probed accuracy of this exact quantization pipeline: 1.13e-2
(gate 2e-2).  Cost model: ~3.8us per eval-pair vs v1's 6.9us per eval.
"""

import os

import numpy as np
from ml_dtypes import bfloat16 as _bf16
from ml_dtypes import float8_e4m3 as _f8np

N_CORES = 8
G = 2          # pipelined sample groups per core
CTX8 = bool(int(os.environ.get("KERNEL_CTX8", "1")))  # fp8 DoubleRow ctx matmuls
OP_LABELS = {}  # instruction name -> human label (for the trace analyzer)
S_H = 0.25     # h2 scale carried in the GLU2 STT scalar


def _build_program(theta0, context, W1, b1, W2, b2, W3, b3, n_steps):
    import concourse.mybir as mybir
    import concourse.tile as tile
    from concourse import bacc

    f32 = mybir.dt.float32
    f32r = mybir.dt.float32r
    bf16 = mybir.dt.bfloat16
    f8 = mybir.dt.float8e4
    DR = mybir.MatmulPerfMode.DoubleRow
    ALU = mybir.AluOpType
    SIGMOID = mybir.ActivationFunctionType.Sigmoid

    B, D = theta0.shape          # 2048, 32
    C = context.shape[1]         # 128
    IN, H2 = W1.shape            # 161, 1024
    H = W2.shape[0]              # 512
    assert H2 == 2 * H and W2.shape[1] == 2 * H and W3.shape == (H, D)
    assert IN == D + 1 + C
    assert B % (N_CORES * G) == 0
    Bs = B // N_CORES            # 256
    Ng = Bs // G                 # 128
    steps = int(n_steps)
    dt = 1.0 / steps
    MJ = H // 128                # 4 column chunks per GLU half
    KCP = MJ // 2                # 2 DoubleRow pairs over the H contraction
    NEV = 4 * steps
    DT2 = D + 2                  # moving rows: theta(32) + t(1) + ones(1)

    b2f = np.asarray(b2, np.float32)
    b3f = np.asarray(b3, np.float32)
    b2nz = bool(np.any(b2f))
    b3nz = bool(np.any(b3f))

    # t value per eval (t = idx * dt/2)
    IOFF = (0, 1, 1, 2)
    TVAL = [(2 * (e // 4) + IOFF[e % 4]) * (dt / 2.0) for e in range(NEV + 1)]

    # ---- host-side layout prep (shared across cores) ----
    W1f = np.asarray(W1, np.float32)
    w1c_h = np.ascontiguousarray(W1f[D + 1:])                    # [128, 1024]
    if CTX8:
        # DoubleRow pairing along the 128 ctx rows: k = plane*64 + p
        w1c8_h = np.ascontiguousarray(
            w1c_h.reshape(2, C // 2, 2 * H).transpose(1, 0, 2))  # [64, 2, 1024]
    w1tb_h = np.concatenate(
        [W1f[0:D + 1], np.asarray(b1, np.float32).reshape(1, 2 * H)], axis=0
    )                                                            # [34, 1024]
    w2_h = np.ascontiguousarray(
        np.asarray(W2, np.float32)
        .reshape(KCP, 2, 128, 2 * H).transpose(2, 0, 1, 3)
        .reshape(128, KCP * 2 * 2 * H)
    )

    W3f = np.asarray(W3, np.float32)

    def drpack(w, ncol):  # [H, ncol] -> [128, KCP, 2, ncol]
        return w.reshape(KCP, 2, 128, ncol).transpose(2, 0, 1, 3)

    # w3 variants scaled by w_e / s_h (w_e in {1, 2})
    w3v_h = np.ascontiguousarray(np.stack(
        [drpack(W3f * (1.0 / S_H), D), drpack(W3f * (2.0 / S_H), D)], axis=3
    ).reshape(128, KCP * 2 * 2 * D))                             # [128, P, pl, v, D]
    # F variants scaled by c_e / s_h (c_e in {dt/2, dt})
    F_h = W3f @ W1f[0:D]                                         # [512, 1024]
    fv_h = np.ascontiguousarray(np.stack(
        [drpack(F_h * (0.5 * dt / S_H), 2 * H), drpack(F_h * (dt / S_H), 2 * H)],
        axis=3,
    ).reshape(128, KCP * 2 * 2 * 2 * H))                         # [128, P, pl, v, 1024]

    # ---- build the bass program (same program on all 8 cores) ----
    nc = bacc.Bacc("TRN2", target_bir_lowering=False)

    if CTX8:
        d_ctxw = nc.dram_tensor("ctxw", [C // 2, 2 * (G * Ng + 2 * H)], f8,
                                kind="ExternalInput")
    else:
        d_ctxw = nc.dram_tensor("ctxw", [C, G * Ng + 2 * H], bf16,
                                kind="ExternalInput")
    d_thw = nc.dram_tensor("thw", [DT2, G * Ng + 2 * H], bf16, kind="ExternalInput")
    d_thF = nc.dram_tensor("thF", [D, Bs], f32, kind="ExternalInput")
    d_w2 = nc.dram_tensor("w2", [128, KCP * 2 * 2 * H], f8, kind="ExternalInput")
    d_w3v = nc.dram_tensor("w3v", [128, KCP * 2 * 2 * D], f8, kind="ExternalInput")
    d_fv = nc.dram_tensor("fv", [128, KCP * 2 * 2 * 2 * H], f8, kind="ExternalInput")
    # bias fallbacks (all-zero in the reference problem): bias values ride as
    # single-row matmul stationaries against the ones row of the moving tile
    d_b2t = (nc.dram_tensor("b2t", [1, 2 * H], bf16, kind="ExternalInput")
             if b2nz else None)
    d_b3r = (nc.dram_tensor("b3r", [1, 2 * D + 2 * 2 * H], bf16, kind="ExternalInput")
             if b3nz else None)
    d_out = nc.dram_tensor("out", [D, Bs], f32, kind="ExternalOutput")

    DBG = bool(int(os.environ.get("KERNEL_DBG", "0")))
    d_dbg = {}
    if DBG:
        for nm, shp in (
            ("dbg_l1b_e0", [128, MJ * Ng]), ("dbg_sg1_e0", [128, MJ * Ng]),
            ("dbg_h1_e0", [128, MJ * Ng]), ("dbg_l2b_e0", [128, MJ * Ng]),
            ("dbg_h2_e0", [128, MJ * Ng]), ("dbg_acc_e0", [D, Ng]),
            ("dbg_l1b_e1", [128, MJ * Ng]), ("dbg_l1a_e1", [128, MJ * Ng]),
        ):
            d_dbg[nm] = nc.dram_tensor(nm, shp, f32, kind="ExternalOutput")

    PSB = int(os.environ.get("KERNEL_PSB", "6"))
    SGB = int(os.environ.get("KERNEL_SGB", "6"))
    HB = int(os.environ.get("KERNEL_HB", "6"))

    with tile.TileContext(nc) as tc:
        with (
            tc.tile_pool(name="const", bufs=1) as cpool,
            tc.tile_pool(name="psb", bufs=PSB, space="PSUM") as pspool,
            tc.tile_pool(name="pss", bufs=1, space="PSUM") as psspool,
            tc.tile_pool(name="sg", bufs=SGB) as sgpool,
            tc.tile_pool(name="h", bufs=HB) as hpool,
        ):
            if CTX8:
                tctxw = cpool.tile([C // 2, 2, G * Ng + 2 * H], f8)
                tctx = [tctxw[:, :, g * Ng:(g + 1) * Ng] for g in range(G)]
            else:
                tctxw = cpool.tile([C, G * Ng + 2 * H], bf16)
                tctx = [tctxw[:, g * Ng:(g + 1) * Ng] for g in range(G)]
            tthw = cpool.tile([DT2, G * Ng + 2 * H], bf16)
            tth = [tthw[:, g * Ng:(g + 1) * Ng] for g in range(G)]
            tthF = cpool.tile([D, G, Ng], f32)
            tw2 = cpool.tile([128, KCP, 2, 2 * H], f8)
            tw3v = cpool.tile([128, KCP, 2, 2, D], f8)
            tF = cpool.tile([128, KCP, 2, 2, 2 * H], f8)
            if b2nz:
                tb2 = cpool.tile([1, 2 * H], bf16)
            if b3nz:
                # cols: [b3 | 2*b3 | (dt/2)*b3@W1th | dt*b3@W1th]
                tb3 = cpool.tile([1, 2 * D + 2 * 2 * H], bf16)
            # one acc bank per group: PSUM start=True is bank-granular, so
            # the accumulators cannot share a bank with anything live
            psacc = [psspool.tile([D, Ng], f32, name=f"acc{g}") for g in range(G)]

            def w1c_col(half, j):
                base = G * Ng + half * H + j * 128
                if CTX8:
                    return tctxw[:, :, base:base + 128]
                return tctxw[:, base:base + 128]

            def w1tb_col(half, j):
                base = G * Ng + half * H + j * 128
                return tthw[:, base:base + 128]

            # startup DMAs: L1-critical tensors first, weights stream behind
            nc.sync.dma_start(tctxw[:], d_ctxw[:])
            nc.sync.dma_start(tthw[:], d_thw[:])
            nc.sync.dma_start(tthF[:], d_thF[:])
            for P in range(KCP):
                nc.sync.dma_start(tw2[:, P, :, :],
                                  d_w2[:, P * 2 * 2 * H:(P + 1) * 2 * 2 * H])
            nc.sync.dma_start(tw3v[:], d_w3v[:])
            nc.sync.dma_start(tF[:], d_fv[:])
            if b2nz:
                nc.sync.dma_start(tb2[:], d_b2t[:])
            if b3nz:
                nc.sync.dma_start(tb3[:], d_b3r[:])

            CUR = ["?"]

            def lab(inst, name):
                try:
                    OP_LABELS[inst.name] = name
                except Exception:
                    pass
                return inst

            def mm(out_ap, lhsT, rhs, start, stop, pm=None):
                lab(nc.tensor.matmul(out_ap, lhsT, rhs, start=start, stop=stop,
                                     perf_mode=pm), CUR[0])

            L1 = {}
            H1out = {}

            def issue_l1(g, close):
                """Pre-issue next eval's static L1 parts: ctx + (theta_s,
                t, b1) matmuls.  close=True ends the accumulation groups
                (step boundary, no F-term); else F matmuls close later."""
                bb = pspool.tile([128, MJ, Ng], f32, tag="bank", name=f"L1b{g}")
                ba = pspool.tile([128, MJ, Ng], f32, tag="bank", name=f"L1a{g}")
                CUR[0] = f"ctxMM.{g}"
                for half, bank in ((1, bb), (0, ba)):
                    for j in range(MJ):
                        # start only on the bank's first MM: a second start
                        # re-marks the whole bank pending-zero
                        mm(bank[:, j, :], w1c_col(half, j), tctx[g][:],
                           start=(j == 0), stop=False,
                           pm=(DR if CTX8 else None))
                CUR[0] = f"thMM.{g}"
                for half, bank in ((1, bb), (0, ba)):
                    for j in range(MJ):
                        mm(bank[:, j, :], w1tb_col(half, j), tth[g][:],
                           start=False, stop=close)
                return bb, ba

            def dbg_dump(nm, ap):
                if DBG and nm in d_dbg:
                    t = cpool.tile([ap.shape[0], int(np.prod(ap.shape[1:]))], f32,
                                   name=nm)
                    nc.scalar.copy(t[:], ap)
                    nc.sync.dma_start(d_dbg[nm][:], t[:])

            def phase1(g, e):
                # sigma1 over the whole b-bank, GLU1 -> fp8 pair tile
                bb, ba = L1[g]
                sg = sgpool.tile([128, MJ, Ng], bf16, tag="sg", name=f"sg1{g}")
                lab(nc.scalar.activation(sg[:], bb[:, :, :], SIGMOID), f"sig1.{g}.{e}")
                h1 = hpool.tile([128, MJ, Ng], f8, tag="h1", name=f"h1{g}")
                lab(nc.vector.scalar_tensor_tensor(h1[:], ba[:, :, :], 1.0, sg[:],
                                                   ALU.mult, ALU.mult), f"glu1.{g}.{e}")
                H1out[g] = h1
                if DBG and g == 0 and e == 0:
                    dbg_dump("dbg_l1b_e0", bb[:, :, :])
                    dbg_dump("dbg_sg1_e0", sg[:])
                    dbg_dump("dbg_h1_e0", h1[:])
                if DBG and g == 0 and e == 1:
                    dbg_dump("dbg_l1b_e1", bb[:, :, :])
                    dbg_dump("dbg_l1a_e1", ba[:, :, :])

            L2banks = {}

            def phase2L(g, e):
                # L2 matmuls + sigma2 (L2a runs behind sigma2 on the PE)
                h1 = H1out[g]
                CUR[0] = f"L2MM.{g}.{e}"
                bb2 = pspool.tile([128, MJ, Ng], f32, tag="bank", name=f"L2b{g}")
                ba2 = pspool.tile([128, MJ, Ng], f32, tag="bank", name=f"L2a{g}")
                for j in range(MJ):
                    for P in range(KCP):
                        mm(bb2[:, j, :], tw2[:, P, :, H + j * 128:H + (j + 1) * 128],
                           h1[:, 2 * P:2 * P + 2, :],
                           start=(j == 0 and P == 0),
                           stop=(P == KCP - 1 and not b2nz), pm=DR)
                if b2nz:  # fallback: bias via ones-row matmuls (b-half)
                    for j in range(MJ):
                        mm(bb2[:, j, :], tb2[:, H + j * 128:H + (j + 1) * 128],
                           tth[g][D + 1:D + 2, :], start=False, stop=True)
                sg2 = sgpool.tile([128, MJ, Ng], bf16, tag="sg", name=f"sg2{g}")
                lab(nc.scalar.activation(sg2[:], bb2[:, :, :], SIGMOID), f"sig2.{g}.{e}")
                for j in range(MJ):
                    for P in range(KCP):
                        mm(ba2[:, j, :], tw2[:, P, :, j * 128:(j + 1) * 128],
                           h1[:, 2 * P:2 * P + 2, :],
                           start=(j == 0 and P == 0),
                           stop=(P == KCP - 1 and not b2nz), pm=DR)
                if b2nz:
                    for j in range(MJ):
                        mm(ba2[:, j, :], tb2[:, j * 128:(j + 1) * 128],
                           tth[g][D + 1:D + 2, :], start=False, stop=True)
                L2banks[g] = (bb2, ba2, sg2)

            def pre_issue(g, e):
                # static L1 parts of eval e (ctx + theta_s + t + b1) — no data
                # deps beyond tth/tctx, so these fill PE dependency-wait gaps.
                # Skipped for e%4==0 (issued in phase2T after the state copy).
                nc.gpsimd.memset(tth[g][D:D + 1, :], float(TVAL[e]))
                L1[g] = issue_l1(g, close=False)

            def phase2T(g, e):
                # GLU2 -> acc matmuls -> F-shortcut (or boundary state update)
                s, ei = divmod(e, 4)
                last = e == NEV - 1
                boundary = ei == 3
                bb2, ba2, sg2 = L2banks[g]
                h2 = hpool.tile([128, MJ, Ng], f8, tag="h2", name=f"h2{g}")
                lab(nc.vector.scalar_tensor_tensor(h2[:], ba2[:, :, :], S_H, sg2[:],
                                                   ALU.mult, ALU.mult), f"glu2.{g}.{e}")
                acc = psacc[g][:]
                v = 0 if ei in (0, 3) else 1

                def acc_mms():
                    # RK4 accumulator: acc += w_e * k_e (pre-scaled W3)
                    CUR[0] = f"accMM.{g}.{e}"
                    for P in range(KCP):
                        mm(acc, tw3v[:, P, :, v, :], h2[:, 2 * P:2 * P + 2, :],
                           start=(ei == 0 and P == 0),
                           stop=(P == KCP - 1 and not b3nz), pm=DR)
                    if b3nz:
                        boff = 0 if ei in (0, 3) else D
                        mm(acc, tb3[:, boff:boff + D],
                           tth[g][D + 1:D + 2, :], start=False, stop=True)

                if boundary:
                    acc_mms()   # thSTT depends on acc: keep it first
                if DBG and g == 0 and e == 0:
                    dbg_dump("dbg_l2b_e0", bb2[:, :, :])
                    dbg_dump("dbg_h2_e0", h2[:])
                if not boundary:
                    # F-shortcut: theta-correction of the NEXT eval's L1 pre-
                    # activations directly from h2 (closes the L1 banks);
                    # b-bank first so sigma1 unblocks before GLU1 needs a-bank
                    CUR[0] = f"FMM.{g}.{e}"
                    fv = 0 if ei < 2 else 1
                    bb, ba = L1[g]
                    for half, bank in ((1, bb), (0, ba)):
                        for j in range(MJ):
                            col = half * H + j * 128
                            for P in range(KCP):
                                mm(bank[:, j, :],
                                   tF[:, P, :, fv, col:col + 128],
                                   h2[:, 2 * P:2 * P + 2, :],
                                   start=False,
                                   stop=(P == KCP - 1 and not b3nz), pm=DR)
                    if b3nz:
                        # c_e * (b3 @ W1theta) correction row via the ones row
                        for half, bank in ((1, bb), (0, ba)):
                            for j in range(MJ):
                                boff = 2 * D + fv * 2 * H + half * H + j * 128
                                mm(bank[:, j, :], tb3[:, boff:boff + 128],
                                   tth[g][D + 1:D + 2, :], start=False, stop=True)
                    acc_mms()  # off the critical chain on fast boundaries
                elif last:
                    nc.vector.scalar_tensor_tensor(
                        tthF[:, g, :], acc, float(dt / 6.0), tthF[:, g, :],
                        ALU.mult, ALU.add)
                else:
                    # step boundary.  Critical chain: acc -> bf16 theta tile
                    # -> theta matmuls -> sigma1(e+1).  The f32 state update
                    # (same inputs) runs behind it, off-chain.
                    lab(nc.vector.scalar_tensor_tensor(
                        tth[g][0:D, :], acc, float(dt / 6.0), tthF[:, g, :],
                        ALU.mult, ALU.add), f"thSTT.{g}.{e}")
                    nc.gpsimd.memset(tth[g][D:D + 1, :], float(TVAL[e + 1]))
                    L1[g] = issue_l1(g, close=True)
                    nc.vector.scalar_tensor_tensor(
                        tthF[:, g, :], acc, float(dt / 6.0), tthF[:, g, :],
                        ALU.mult, ALU.add)

            # ---- prologue: first-eval static parts for both groups ----
            L1[0] = issue_l1(0, close=True)
            L1[1] = issue_l1(1, close=True)

            # ---- slot walk: B lags A by a quarter period.  Per slot the
            # engine streams are ACT [s1A, s2B, s2A, s1B], DVE [g1A, g2B,
            # g2A, g1B], PE [L2B, preB', L2A, accB+FB, preA', accA+FA] so
            # neither group's chain waits on the other's long segments. ----
            for e in range(NEV):
                phase1(0, e)
                if e > 0:
                    phase2L(1, e - 1)
                    if (e - 1) % 4 != 3:
                        pre_issue(1, e)
                phase2L(0, e)
                if e > 0:
                    phase2T(1, e - 1)
                if e % 4 != 3:
                    pre_issue(0, e + 1)
                phase2T(0, e)
                phase1(1, e)
            phase2L(1, NEV - 1)
            phase2T(1, NEV - 1)

            nc.sync.dma_start(d_out[:], tthF[:, :, :])

    # ---- per-core input maps ----
    w1c_b = w1c_h.astype(_bf16)
    w1tb_b = w1tb_h.astype(_bf16)
    w2_q = w2_h.astype(_f8np)
    w3v_q = w3v_h.astype(_f8np)
    fv_q = fv_h.astype(_f8np)
    if b2nz:
        b2t_h = np.ascontiguousarray(b2f.reshape(1, 2 * H)).astype(_bf16)
    if b3nz:
        b3w = (b3f @ W1f[0:D]).reshape(1, 2 * H)
        b3r_h = np.concatenate(
            [b3f.reshape(1, D), 2.0 * b3f.reshape(1, D),
             0.5 * dt * b3w, dt * b3w], axis=1).astype(_bf16)
    in_maps = []
    for c in range(N_CORES):
        sl = slice(c * Bs, (c + 1) * Bs)
        th_T = np.ascontiguousarray(np.asarray(theta0[sl], np.float32).T)  # [32,256]
        ctx_T = np.ascontiguousarray(np.asarray(context[sl], np.float32).T)  # [128,256]
        thg = []
        for g in range(G):
            t34 = np.zeros((DT2, Ng), np.float32)
            t34[0:D] = th_T[:, g * Ng:(g + 1) * Ng]
            t34[D] = 0.0          # t row (t=0 at start)
            t34[D + 1] = 1.0      # ones row
            thg.append(t34)
        thw = np.ascontiguousarray(np.concatenate(
            [np.concatenate(thg, axis=1).astype(_bf16), w1tb_b], axis=1))
        if CTX8:
            ctx_dr = ctx_T.reshape(2, C // 2, Bs).transpose(1, 0, 2)
            ctxw = np.ascontiguousarray(np.concatenate(
                [ctx_dr, w1c8_h], axis=2).reshape(C // 2, -1)).astype(_f8np)
        else:
            ctxw = np.ascontiguousarray(np.concatenate(
                [ctx_T.astype(_bf16), w1c_b], axis=1))
        m = {
            "ctxw": ctxw,
            "thw": thw,
            "thF": th_T,
            "w2": w2_q,
            "w3v": w3v_q,
            "fv": fv_q,
        }
        if b2nz:
            m["b2t"] = b2t_h
        if b3nz:
            m["b3r"] = b3r_h
        in_maps.append(m)

    return nc, in_maps


def _build_and_run(theta0, context, W1, b1, W2, b2, W3, b3, n_steps):
    from concourse.bass_utils import run_bass_kernel_spmd

    nc, in_maps = _build_program(theta0, context, W1, b1, W2, b2, W3, b3, n_steps)
    nc.finalize()
    res = run_bass_kernel_spmd(
        nc,
        in_maps,
        core_ids=list(range(N_CORES)),
        trace=bool(int(os.environ.get("KERNEL_TRACE", "0"))),
    )
    _build_and_run.last_results = res

    out = np.concatenate([r["out"].T for r in res.results], axis=0)
    return np.ascontiguousarray(out.astype(np.float32))


def kernel(theta0, context, W1, b1, W2, b2, W3, b3, n_steps):
    return _build_and_run(
        np.asarray(theta0), np.asarray(context), W1, b1, W2, b2, W3, b3, n_steps
    )
